# revision 19
# baseline (speedup 1.0000x reference)
"""Distributed Trainium2 (8 NeuronCores) kernel for a 2-layer GraphSAGE
autoencoder (mean aggregation) — nn_AnomalyDetector.

Strategy (vertex-cut by destination owner):
  - Nodes are sharded contiguously across 8 cores (12500 each, padded to
    12544 = 98 tiles x 128 slots).  Each core owns the edges whose dst it
    owns; segment-sum is local.
  - Layer 1 gathers rows of the (replicated) x table with dma_gather
    (int16 indices -> 4 overlapping 32768-row table segments; per-edge
    segment assignment is waterfilled per destination tile, with per-run
    static block quotas shared by all 8 cores so the SPMD graph is
    identical).
  - Segment-sum is a PE matmul against a one-hot matrix built on DVE
    (is_equal vs an iota row), accumulated in PSUM per destination tile.
    Per-edge 1/deg scaling is folded into the gathered rows, so padding
    slots (scale 0) contribute nothing.
  - Feat-major layouts throughout: aggT [f,n] -> hT [h,n] -> p [n,l]
    (node-major via operand swap) + s2T [l,n].  p is AllGathered to the
    full [100352, 64] table for the layer-2 gather.  Outputs xrecT
    [128, 12544] and zT [64, 12544] are transposed/trimmed on the host.
"""

import os
import sys

for _p in ("/opt/trn_rl_repo", "/root/.axon_site/_ro/trn_rl_repo"):
    if os.path.isdir(_p) and _p not in sys.path:
        sys.path.append(_p)

import numpy as np
import ml_dtypes

BF = ml_dtypes.bfloat16

import concourse.bass as bass
import concourse.mybir as mybir
from concourse import bacc
from concourse.tile import TileContext
from concourse.bass_utils import run_bass_kernel_spmd
from concourse.library_config import mlp

F32 = mybir.dt.float32
BF16 = mybir.dt.bfloat16
I16 = mybir.dt.int16

NCORES = 8
P = 128          # partitions / block size / tile node count
SEG_SPAN = 32768  # int16 index reach
NSEG = 4
SCHUNK = 16      # blocks per one-hot build
CALL_BLKS = 64   # max blocks per dma_gather call


# ----------------------------------------------------------------- config

class Cfg:
    def __init__(self, n, e, in_ch, hid_ch, lat_ch, npc, gsize):
        assert n % NCORES == 0 or True
        self.N, self.E = n, e
        self.IN, self.HID, self.LAT = in_ch, hid_ch, lat_ch
        self.NPC = npc                        # real nodes per core
        self.NPCP = -(-npc // P) * P          # padded
        self.TPC = self.NPCP // P             # tiles per core
        self.GSIZE = gsize                    # tiles per group
        assert self.TPC % gsize == 0
        self.NG = self.TPC // gsize
        # table sizes
        self.XROWS = n                        # layer-1 table rows
        self.PROWS = self.NPCP * NCORES       # layer-2 table rows

    def seg_bases(self, rows):
        if rows <= SEG_SPAN:
            return [0] * NSEG
        step = -(-(rows - SEG_SPAN) // (NSEG - 1))
        return [min(s * step, rows - SEG_SPAN) for s in range(NSEG)]


# ---------------------------------------------------------- host schedule

def _intervals(bases, rows):
    """Split [0, rows) into intervals with constant admissible-segment sets.
    Returns (bounds, adm) where bounds has len nivl+1 and adm[i] is the
    ordered list of admissible segs for interval i."""
    lo = np.array(bases)
    hi = np.minimum(lo + SEG_SPAN, rows)
    cuts = np.unique(np.concatenate([lo, hi, [0, rows]]))
    cuts = cuts[(cuts >= 0) & (cuts <= rows)]
    adm = []
    bounds = []
    for i in range(len(cuts) - 1):
        a, b = cuts[i], cuts[i + 1]
        if a == b:
            continue
        mid = (a + b) // 2
        segs = [s for s in range(NSEG) if lo[s] <= mid < hi[s]]
        assert segs, f"uncovered interval [{a},{b})"
        bounds.append(a)
        adm.append(segs)
    bounds.append(rows)
    return np.array(bounds), adm


def _waterfill_counts(counts, adm):
    """counts[i] edges in interval i with admissible segs adm[i].
    Returns take[i][s] = how many of interval i go to seg s.

    Fills each non-final segment up to a multiple of P (block grid) so the
    per-(tile,seg) quotas waste as little block padding as possible; the
    remainder lands in the last admissible segment."""
    total = int(counts.sum())
    # grid target: fill each seg to the largest multiple of P that the
    # average load reaches (at least P)
    target = max(P, (total // NSEG) // P * P)
    loads = np.zeros(NSEG, np.int64)
    take = []
    for i, n in enumerate(counts):
        n = int(n)
        segs = adm[i]
        tk = {}
        for s in segs[:-1]:
            a = int(np.clip(target - loads[s], 0, n))
            tk[s] = a
            loads[s] += a
            n -= a
        tk[segs[-1]] = n
        loads[segs[-1]] += n
        take.append(tk)
    return take


def build_layer_schedule(cfg, src_by_core, dst_local_by_core, deginv_by_core,
                         table_rows):
    """Compute the SPMD-static schedule + per-core slot arrays for one layer.

    src_by_core[c]: global table row per edge (int64)
    dst_local_by_core[c]: local node id (0..NPC-1) per edge
    deginv_by_core[c]: f32 scale per edge
    Returns (sched, percore) where sched is shared and percore is a list of
    dicts with device-ready arrays.
    """
    bases = cfg.seg_bases(table_rows)
    TPC = cfg.TPC
    bounds, adm = _intervals(bases, table_rows)
    nivl = len(adm)

    # per (core, tile, seg) edge index lists
    per_cts = [[[None] * NSEG for _ in range(TPC)] for _ in range(NCORES)]
    for c in range(NCORES):
        src = src_by_core[c]
        dstl = dst_local_by_core[c]
        tile = (dstl // P).astype(np.int64)
        iv = np.searchsorted(bounds, src, side="right") - 1
        order = np.lexsort((iv, tile))
        # counts matrix [TPC, nivl]
        cnt = np.zeros((TPC, nivl), np.int64)
        np.add.at(cnt, (tile, iv), 1)
        # prefix offsets of each (t, iv) run in `order`
        run_off = np.zeros((TPC, nivl), np.int64)
        flat = cnt.reshape(-1).cumsum()
        run_off.reshape(-1)[1:] = flat[:-1]
        for t in range(TPC):
            take = _waterfill_counts(cnt[t], adm)
            buckets = {s: [] for s in range(NSEG)}
            for i in range(nivl):
                o = int(run_off[t, i])
                n = int(cnt[t, i])
                pos = 0
                for s, k in take[i].items():
                    if k:
                        buckets[s].append(order[o + pos:o + pos + k])
                        pos += k
            for s in range(NSEG):
                per_cts[c][t][s] = (np.concatenate(buckets[s])
                                    if buckets[s] else np.empty(0, np.int64))

    # static quotas (blocks) per (tile, seg): max over cores
    quota = np.zeros((TPC, NSEG), np.int64)
    for t in range(TPC):
        for s in range(NSEG):
            mx = max(len(per_cts[c][t][s]) for c in range(NCORES))
            quota[t, s] = -(-mx // P)

    # slot layout: [group][seg][tile in group][quota blocks]
    blocks = []           # list of (tile, seg) per block, in slot order
    calls = []            # list of (seg, blk_start, nblk)
    for g in range(cfg.NG):
        tiles = range(g * cfg.GSIZE, (g + 1) * cfg.GSIZE)
        for s in range(NSEG):
            run_start = len(blocks)
            for t in tiles:
                for _ in range(quota[t, s]):
                    blocks.append((t, s))
            run_len = len(blocks) - run_start
            b0 = run_start
            while run_len > 0:
                nb = min(run_len, CALL_BLKS)
                calls.append((s, b0, nb))
                b0 += nb
                run_len -= nb
    totblk = len(blocks)
    sched = dict(bases=bases, quota=quota, blocks=blocks, calls=calls,
                 totblk=totblk)

    # per-core arrays
    percore = []
    for c in range(NCORES):
        idx_rel = np.zeros(totblk * P, np.int64)
        dstoff = np.zeros(totblk * P, np.float32)
        scale = np.zeros(totblk * P, np.float32)
        # fill slots tile/seg cell by cell
        cell_cursor = {}
        b_of_cell = {}
        pos = 0
        for bi, (t, s) in enumerate(blocks):
            b_of_cell.setdefault((t, s), []).append(bi)
        for (t, s), bl in b_of_cell.items():
            e = per_cts[c][t][s]
            slots = np.concatenate([np.arange(b * P, (b + 1) * P) for b in bl])
            k = len(e)
            assert k <= len(slots)
            src = src_by_core[c][e]
            idx_rel[slots[:k]] = src - bases[s]
            dstoff[slots[:k]] = (dst_local_by_core[c][e] % P).astype(np.float32)
            scale[slots[:k]] = deginv_by_core[c][e]
            # padding slots keep idx_rel 0 (valid row), scale 0
        assert idx_rel.min() >= 0 and idx_rel.max() < SEG_SPAN

        # [128, totblk] layouts (slot -> p = slot%128, k = slot//128)
        dstoff_t = dstoff.reshape(totblk, P).T.copy()
        scale_t = scale.reshape(totblk, P).T.copy()

        # int16 idx array packed per call: for call (s, b0, nb):
        # within-call j -> [j%16, col j//16], replicated over 8 groups,
        # stored at columns [b0*8, (b0+nb)*8)
        idx16 = np.zeros((P, totblk * 8), np.int16)
        for (s, b0, nb) in calls:
            n = nb * P
            j = np.arange(n)
            vals = idx_rel[b0 * P: b0 * P + n].astype(np.int16)
            cols = b0 * 8 + j // 16
            for grp in range(8):
                idx16[grp * 16 + (j % 16), cols] = vals
        percore.append(dict(idx16=idx16, dstoff=dstoff_t, scale=scale_t))
    return sched, percore


# ------------------------------------------------------------- bass build

def emit_gather_segsum(nc, tc, pools, cfg, sched, elem, tbl, idx_res, doff_res,
                       scal_res, psum_pool, psum_shape, evict_fn, qstate,
                       iota_t, gdt=F32, post_call_hooks=None):
    """Emit one layer's gather + scaled one-hot segment-sum.

    evict_fn(t, psum_tile) is called once per tile after its accumulation
    completes.
    """
    gpool, spool = pools
    blocks, calls, quota = sched["blocks"], sched["calls"], sched["quota"]
    bases = sched["bases"]
    # first/last block index per tile
    first_blk, last_blk = {}, {}
    for bi, (t, s) in enumerate(blocks):
        first_blk.setdefault(t, bi)
        last_blk[t] = bi
    psum_tiles = {}

    for ci, (s, b0, nb) in enumerate(calls):
        n_idx = nb * P
        G = gpool.tile([P, CALL_BLKS * elem], gdt, tag="G")
        lo = bases[s]
        hi = min(lo + SEG_SPAN, tbl.shape[0])
        nc.gpsimd.dma_gather(
            G[:].rearrange("p (k d) -> p k d", d=elem)[:, :nb, :],
            tbl[lo:hi, :],
            idx_res[:, b0 * 8:(b0 + nb) * 8],
            n_idx, n_idx, elem,
            single_packet=False,
            queue_num=qstate[0],
        )
        qstate[0] = (qstate[0] + 1) % 4
        # per-edge scale (also zeroes padding slots); bf16 for the PE
        GS = gpool.tile([P, CALL_BLKS * elem], BF16, tag="GS")
        G3 = G[:].rearrange("p (k d) -> p k d", d=elem)[:, :nb, :]
        GS3 = GS[:].rearrange("p (k d) -> p k d", d=elem)[:, :nb, :]
        nc.vector.tensor_tensor(
            out=GS3, in0=G3,
            in1=doff_scale_bcast(scal_res, b0, nb, elem),
            op=mybir.AluOpType.mult,
        )
        # one-hot chunks + matmuls
        for sc0 in range(0, nb, SCHUNK):
            scn = min(SCHUNK, nb - sc0)
            S = spool.tile([P, SCHUNK * P], BF16, tag="S")
            S3 = S[:].rearrange("p (k n) -> p k n", n=P)[:, :scn, :]
            nc.vector.tensor_tensor(
                out=S3,
                in0=doff_scale_bcast(doff_res, b0 + sc0, scn, P),
                in1=iota_t[:].rearrange("p (a n) -> p a n", a=1)
                    .to_broadcast([P, scn, P]),
                op=mybir.AluOpType.is_equal,
            )
            for k in range(scn):
                bi = b0 + sc0 + k
                t, _s = blocks[bi]
                if t not in psum_tiles:
                    seg_psum = psum_pool.tile(list(psum_shape), F32, tag="seg")
                    psum_tiles[t] = seg_psum
                nc.tensor.matmul(
                    psum_tiles[t][:],
                    lhsT=GS[:, (sc0 + k) * elem:(sc0 + k + 1) * elem],
                    rhs=S[:, k * P:(k + 1) * P],
                    start=(bi == first_blk[t]),
                    stop=(bi == last_blk[t]),
                )
                if bi == last_blk[t]:
                    evict_fn(t, psum_tiles.pop(t))
        if post_call_hooks and ci in post_call_hooks:
            post_call_hooks[ci]()


def doff_scale_bcast(res, k0, nb, width):
    return res[:, k0:k0 + nb].to_broadcast([P, nb, width])


def build_bass(cfg, sched1, sched2, compile_=True):
    nc = bacc.Bacc("TRN2", target_bir_lowering=False, debug=False,
                   num_devices=NCORES, num_swdge_queues=4)
    IN, HID, LAT = cfg.IN, cfg.HID, cfg.LAT
    NPCP, TPC = cfg.NPCP, cfg.TPC

    x_tbl = nc.declare_dram_parameter("x_tbl", [cfg.XROWS, IN], BF16, isOutput=False)
    xT_c = nc.declare_dram_parameter("xT_c", [IN, NPCP], F32, isOutput=False)
    idx1 = nc.declare_dram_parameter("idx1", [P, sched1["totblk"] * 8], I16, isOutput=False)
    doff1 = nc.declare_dram_parameter("doff1", [P, sched1["totblk"]], BF16, isOutput=False)
    scal1 = nc.declare_dram_parameter("scal1", [P, sched1["totblk"]], BF16, isOutput=False)
    idx2 = nc.declare_dram_parameter("idx2", [P, sched2["totblk"] * 8], I16, isOutput=False)
    doff2 = nc.declare_dram_parameter("doff2", [P, sched2["totblk"]], BF16, isOutput=False)
    scal2 = nc.declare_dram_parameter("scal2", [P, sched2["totblk"]], BF16, isOutput=False)
    w1lt = nc.declare_dram_parameter("w1lt", [IN, HID], F32, isOutput=False)
    w1rt = nc.declare_dram_parameter("w1rt", [IN, HID], F32, isOutput=False)
    b1 = nc.declare_dram_parameter("b1", [HID, 1], F32, isOutput=False)
    w2lt = nc.declare_dram_parameter("w2lt", [HID, LAT], F32, isOutput=False)
    w2rt = nc.declare_dram_parameter("w2rt", [HID, LAT], F32, isOutput=False)
    b2 = nc.declare_dram_parameter("b2", [LAT, 1], F32, isOutput=False)
    wdt = nc.declare_dram_parameter("wdt", [LAT, IN], F32, isOutput=False)
    bd = nc.declare_dram_parameter("bd", [IN, 1], F32, isOutput=False)
    iota_in = nc.declare_dram_parameter("iota_in", [P, P], BF16, isOutput=False)

    zT_out = nc.declare_dram_parameter("zT_out", [LAT, NPCP], F32, isOutput=True)
    xrT_out = nc.declare_dram_parameter("xrT_out", [IN, NPCP], F32, isOutput=True)

    p_self_a = nc.dram_tensor("p_self_a", [NPCP // 2, LAT], F32)
    p_self_b = nc.dram_tensor("p_self_b", [NPCP // 2, LAT], F32)
    p_full = nc.dram_tensor("p_full", [cfg.PROWS, LAT], F32, addr_space="Shared")
    s2_dram = nc.dram_tensor("s2_dram", [LAT, NPCP], F32)

    with TileContext(nc) as tc:
        with tc.tile_pool(name="const", bufs=1) as cpool, \
             tc.tile_pool(name="gpool", bufs=4) as gpool, \
             tc.tile_pool(name="spool", bufs=4) as spool, \
             tc.tile_pool(name="meta", bufs=1) as mpool, \
             tc.tile_pool(name="stage", bufs=3) as stpool, \
             tc.tile_pool(name="psum", bufs=8, space="PSUM") as psum_pool:

            nc.gpsimd.load_library(mlp)

            # constants
            iota_t = cpool.tile([P, P], BF16)
            nc.sync.dma_start(out=iota_t[:], in_=iota_in[:, :])
            w1lt_t = cpool.tile([IN, HID], F32)
            nc.sync.dma_start(out=w1lt_t[:], in_=w1lt[:, :])
            w1rt_t = cpool.tile([IN, HID], F32)
            nc.sync.dma_start(out=w1rt_t[:], in_=w1rt[:, :])
            b1_t = cpool.tile([HID, 1], F32)
            nc.sync.dma_start(out=b1_t[:], in_=b1[:, :])
            w2lt_t = cpool.tile([HID, LAT], F32)
            nc.sync.dma_start(out=w2lt_t[:], in_=w2lt[:, :])
            w2rt_t = cpool.tile([HID, LAT], F32)
            nc.sync.dma_start(out=w2rt_t[:], in_=w2rt[:, :])
            b2_t = cpool.tile([LAT, 1], F32)
            nc.sync.dma_start(out=b2_t[:], in_=b2[:, :])
            wdt_t = cpool.tile([LAT, IN], F32)
            nc.sync.dma_start(out=wdt_t[:], in_=wdt[:, :])
            bd_t = cpool.tile([IN, 1], F32)
            nc.sync.dma_start(out=bd_t[:], in_=bd[:, :])

            # resident metadata, layer 1
            idx1_t = mpool.tile([P, sched1["totblk"] * 8], I16, tag="idx")
            nc.sync.dma_start(out=idx1_t[:], in_=idx1[:, :])
            doff1_t = mpool.tile([P, sched1["totblk"]], BF16)
            nc.sync.dma_start(out=doff1_t[:], in_=doff1[:, :])
            scal1_t = mpool.tile([P, sched1["totblk"]], BF16)
            nc.sync.dma_start(out=scal1_t[:], in_=scal1[:, :])

            qstate = [0]
            half = NPCP // 2

            def emit_ag1():
                nc.gpsimd.collective_compute(
                    "AllGather", mybir.AluOpType.bypass,
                    replica_groups=[list(range(NCORES))],
                    ins=[p_self_a.ap().opt()],
                    outs=[p_full[0:half * NCORES, :].opt()],
                )

            # call index after which every tile < TPC//2 has been evicted
            last_half_blk = max(bi for bi, (t, s) in enumerate(sched1["blocks"])
                                if t < TPC // 2)
            ag1_call = next(ci for ci, (s, b0, nb) in enumerate(sched1["calls"])
                            if b0 + nb > last_half_blk)
            hooks1 = {ag1_call: emit_ag1}

            def evict1(t, psum_t):
                c0, c1 = t * P, (t + 1) * P
                aggT = stpool.tile([IN, P], F32, tag="aggT")
                nc.vector.tensor_copy(out=aggT[:], in_=psum_t[:])
                xT_t = stpool.tile([IN, P], F32, tag="xTt")
                nc.sync.dma_start(out=xT_t[:], in_=xT_c[:, c0:c1])
                hps = psum_pool.tile([HID, P], F32, tag="seg")
                nc.tensor.matmul(hps[:], lhsT=w1lt_t[:], rhs=aggT[:],
                                 start=True, stop=False)
                nc.tensor.matmul(hps[:], lhsT=w1rt_t[:], rhs=xT_t[:],
                                 start=False, stop=True)
                hT = stpool.tile([HID, P], F32, tag="hT")
                nc.scalar.activation(hT[:], hps[:],
                                     mybir.ActivationFunctionType.Relu,
                                     bias=b1_t[:])
                pns = psum_pool.tile([P, LAT], F32, tag="seg")
                nc.tensor.matmul(pns[:], lhsT=hT[:], rhs=w2lt_t[:],
                                 start=True, stop=True)
                s2s = psum_pool.tile([LAT, P], F32, tag="seg")
                nc.tensor.matmul(s2s[:], lhsT=w2rt_t[:], rhs=hT[:],
                                 start=True, stop=True)
                pn = stpool.tile([P, LAT], F32, tag="pn")
                nc.vector.tensor_copy(out=pn[:], in_=pns[:])
                s2 = stpool.tile([LAT, P], F32, tag="s2")
                nc.vector.tensor_scalar_add(s2[:], s2s[:], b2_t[:])
                if c1 <= half:
                    nc.sync.dma_start(out=p_self_a[c0:c1, :], in_=pn[:])
                elif c0 >= half:
                    nc.sync.dma_start(out=p_self_b[c0 - half:c1 - half, :], in_=pn[:])
                else:
                    nc.sync.dma_start(out=p_self_a[c0:half, :], in_=pn[:half - c0, :])
                    nc.sync.dma_start(out=p_self_b[0:c1 - half, :], in_=pn[half - c0:, :])
                nc.sync.dma_start(out=s2_dram[:, c0:c1], in_=s2[:])

            emit_gather_segsum(nc, tc, (gpool, spool), cfg, sched1, IN,
                               x_tbl, idx1_t[:], doff1_t[:], scal1_t[:],
                               psum_pool, (IN, P), evict1, qstate, iota_t,
                               gdt=BF16, post_call_hooks=hooks1)

            # second-half AllGather (first was emitted mid-layer-1)
            nc.gpsimd.collective_compute(
                "AllGather", mybir.AluOpType.bypass,
                replica_groups=[list(range(NCORES))],
                ins=[p_self_b.ap().opt()],
                outs=[p_full[half * NCORES:, :].opt()],
            )

            # resident metadata, layer 2
            idx2_t = mpool.tile([P, sched2["totblk"] * 8], I16, tag="idx")
            nc.sync.dma_start(out=idx2_t[:], in_=idx2[:, :])
            doff2_t = mpool.tile([P, sched2["totblk"]], BF16)
            nc.sync.dma_start(out=doff2_t[:], in_=doff2[:, :])
            scal2_t = mpool.tile([P, sched2["totblk"]], BF16)
            nc.sync.dma_start(out=scal2_t[:], in_=scal2[:, :])

            def evict2(t, psum_t):
                c0, c1 = t * P, (t + 1) * P
                s2t = stpool.tile([LAT, P], F32, tag="s2t")
                nc.sync.dma_start(out=s2t[:], in_=s2_dram[:, c0:c1])
                zT = stpool.tile([LAT, P], F32, tag="zT")
                nc.vector.tensor_add(out=zT[:], in0=psum_t[:], in1=s2t[:])
                xrp = psum_pool.tile([IN, P], F32, tag="seg")
                nc.tensor.matmul(xrp[:], lhsT=wdt_t[:], rhs=zT[:],
                                 start=True, stop=True)
                xr = stpool.tile([IN, P], F32, tag="xr")
                nc.vector.tensor_scalar_add(xr[:], xrp[:], bd_t[:])
                nc.sync.dma_start(out=zT_out[:, c0:c1], in_=zT[:])
                nc.sync.dma_start(out=xrT_out[:, c0:c1], in_=xr[:])

            emit_gather_segsum(nc, tc, (gpool, spool), cfg, sched2, LAT,
                               p_full, idx2_t[:], doff2_t[:], scal2_t[:],
                               psum_pool, (LAT, P), evict2, qstate, iota_t)

    if compile_:
        nc.compile()
    return nc


# ------------------------------------------------------------ host driver

def prepare(x, edge_index, W1_l, b1_, W1_r, W2_l, b2_, W2_r, W_dec, b_dec,
            cfg):
    src = np.asarray(edge_index[0], np.int64)
    dst = np.asarray(edge_index[1], np.int64)
    N, E = cfg.N, cfg.E
    deg = np.bincount(dst, minlength=N).astype(np.float32)
    deginv = 1.0 / np.maximum(deg, 1.0)

    owner = dst // cfg.NPC
    src_by, dstl_by, dgi_by, src2_by = [], [], [], []
    for c in range(NCORES):
        m = np.nonzero(owner == c)[0]
        src_by.append(src[m])
        dstl_by.append(dst[m] - c * cfg.NPC)
        dgi_by.append(deginv[dst[m]])
        so = src[m] // cfg.NPC
        slot = src[m] - so * cfg.NPC
        half = cfg.NPCP // 2
        src2 = np.where(slot < half,
                        so * half + slot,
                        NCORES * half + so * half + (slot - half))
        src2_by.append(src2)

    sched1, pc1 = build_layer_schedule(cfg, src_by, dstl_by, dgi_by, cfg.XROWS)
    sched2, pc2 = build_layer_schedule(cfg, src2_by, dstl_by, dgi_by, cfg.PROWS)

    iota = np.broadcast_to(np.arange(P), (P, P)).astype(BF)
    shared = dict(
        x_tbl=np.ascontiguousarray(np.asarray(x, np.float32).astype(BF)),
        w1lt=np.ascontiguousarray(W1_l.T, np.float32),
        w1rt=np.ascontiguousarray(W1_r.T, np.float32),
        b1=np.ascontiguousarray(b1_.reshape(-1, 1), np.float32),
        w2lt=np.ascontiguousarray(W2_l.T, np.float32),
        w2rt=np.ascontiguousarray(W2_r.T, np.float32),
        b2=np.ascontiguousarray(b2_.reshape(-1, 1), np.float32),
        wdt=np.ascontiguousarray(W_dec.T, np.float32),
        bd=np.ascontiguousarray(b_dec.reshape(-1, 1), np.float32),
        iota_in=iota,
    )
    in_maps = []
    for c in range(NCORES):
        xs = np.zeros((cfg.IN, cfg.NPCP), np.float32)
        xs[:, :cfg.NPC] = x[c * cfg.NPC:(c + 1) * cfg.NPC].T
        m = dict(shared)
        m.update(
            xT_c=xs,
            idx1=pc1[c]["idx16"], doff1=pc1[c]["dstoff"].astype(BF),
            scal1=pc1[c]["scale"].astype(BF),
            idx2=pc2[c]["idx16"], doff2=pc2[c]["dstoff"].astype(BF),
            scal2=pc2[c]["scale"].astype(BF),
        )
        in_maps.append(m)
    return sched1, sched2, in_maps


def assemble(results, cfg):
    xr = np.empty((cfg.N, cfg.IN), np.float32)
    z = np.empty((cfg.N, cfg.LAT), np.float32)
    for c in range(NCORES):
        xr[c * cfg.NPC:(c + 1) * cfg.NPC] = results[c]["xrT_out"][:, :cfg.NPC].T
        z[c * cfg.NPC:(c + 1) * cfg.NPC] = results[c]["zT_out"][:, :cfg.NPC].T
    return xr, z


def kernel(x, edge_index, W1_l, b1_l, W1_r, W2_l, b2_l, W2_r, W_dec, b_dec,
           _trace=False):
    x = np.asarray(x)
    edge_index = np.asarray(edge_index)
    n, e = x.shape[0], edge_index.shape[1]
    cfg = Cfg(n, e, x.shape[1], W1_l.shape[0], W2_l.shape[0],
              n // NCORES, gsize=7 if n == 100000 else 2)
    sched1, sched2, in_maps = prepare(
        x, edge_index, np.asarray(W1_l), np.asarray(b1_l), np.asarray(W1_r),
        np.asarray(W2_l), np.asarray(b2_l), np.asarray(W2_r),
        np.asarray(W_dec), np.asarray(b_dec), cfg)

    nc = build_bass(cfg, sched1, sched2)
    res = run_bass_kernel_spmd(nc, in_maps, core_ids=list(range(NCORES)),
                               trace=_trace)
    xr, z = assemble(res.results, cfg)
    kernel.last_exec_ns = res.exec_time_ns
    return (xr, z)


# revision 24
# speedup vs baseline: 1.0179x; 1.0179x over previous
"""Distributed Trainium2 (8 NeuronCores) kernel for a 2-layer GraphSAGE
autoencoder (mean aggregation) — nn_AnomalyDetector.

Strategy (vertex-cut by destination owner):
  - Nodes are sharded contiguously across 8 cores (12500 each, padded to
    12544 = 98 tiles x 128 slots).  Each core owns the edges whose dst it
    owns; segment-sum is local.
  - Layer 1 gathers rows of the (replicated) x table with dma_gather
    (int16 indices -> 4 overlapping 32768-row table segments; per-edge
    segment assignment is waterfilled per destination tile, with per-run
    static block quotas shared by all 8 cores so the SPMD graph is
    identical).
  - Segment-sum is a PE matmul against a one-hot matrix built on DVE
    (is_equal vs an iota row), accumulated in PSUM per destination tile.
    Per-edge 1/deg scaling is folded into the gathered rows, so padding
    slots (scale 0) contribute nothing.
  - Feat-major layouts throughout: aggT [f,n] -> hT [h,n] -> p [n,l]
    (node-major via operand swap) + s2T [l,n].  p is AllGathered to the
    full [100352, 64] table for the layer-2 gather.  Outputs xrecT
    [128, 12544] and zT [64, 12544] are transposed/trimmed on the host.
"""

import os
import sys

for _p in ("/opt/trn_rl_repo", "/root/.axon_site/_ro/trn_rl_repo"):
    if os.path.isdir(_p) and _p not in sys.path:
        sys.path.append(_p)

import numpy as np
import ml_dtypes

BF = ml_dtypes.bfloat16

import concourse.bass as bass
import concourse.mybir as mybir
from concourse import bacc
from concourse.tile import TileContext
from concourse.bass_utils import run_bass_kernel_spmd
from concourse.library_config import mlp

F32 = mybir.dt.float32
BF16 = mybir.dt.bfloat16
I16 = mybir.dt.int16

NCORES = 8
P = 128          # partitions / block size / tile node count
SEG_SPAN = 32768  # int16 index reach
NSEG = 4
SCHUNK = 16      # blocks per one-hot build
CALL_BLKS = 64   # max blocks per dma_gather call


# ----------------------------------------------------------------- config

class Cfg:
    def __init__(self, n, e, in_ch, hid_ch, lat_ch, npc, gsize):
        assert n % NCORES == 0 or True
        self.N, self.E = n, e
        self.IN, self.HID, self.LAT = in_ch, hid_ch, lat_ch
        self.NPC = npc                        # real nodes per core
        self.NPCP = -(-npc // P) * P          # padded
        self.TPC = self.NPCP // P             # tiles per core
        self.GSIZE = gsize                    # tiles per group
        assert self.TPC % gsize == 0
        self.NG = self.TPC // gsize
        # table sizes
        self.XROWS = n                        # layer-1 table rows
        self.PROWS = self.NPCP * NCORES       # layer-2 table rows

    def seg_bases(self, rows, nseg=NSEG):
        if nseg == 1 or rows <= SEG_SPAN:
            return [0] * nseg
        step = -(-(rows - SEG_SPAN) // (nseg - 1))
        return [min(s * step, rows - SEG_SPAN) for s in range(nseg)]


# ---------------------------------------------------------- host schedule

def _intervals(bases, rows):
    """Split [0, rows) into intervals with constant admissible-segment sets.
    Returns (bounds, adm) where bounds has len nivl+1 and adm[i] is the
    ordered list of admissible segs for interval i."""
    nseg = len(bases)
    lo = np.array(bases)
    hi = np.minimum(lo + SEG_SPAN, rows)
    cuts = np.unique(np.concatenate([lo, hi, [0, rows]]))
    cuts = cuts[(cuts >= 0) & (cuts <= rows)]
    adm = []
    bounds = []
    for i in range(len(cuts) - 1):
        a, b = cuts[i], cuts[i + 1]
        if a == b:
            continue
        mid = (a + b) // 2
        segs = [s for s in range(nseg) if lo[s] <= mid < hi[s]]
        assert segs, f"uncovered interval [{a},{b})"
        bounds.append(a)
        adm.append(segs)
    bounds.append(rows)
    return np.array(bounds), adm


def _waterfill_counts(counts, adm, nseg=NSEG):
    """counts[i] edges in interval i with admissible segs adm[i].
    Returns take[i][s] = how many of interval i go to seg s.

    Fills each non-final segment up to a multiple of P (block grid) so the
    per-(tile,seg) quotas waste as little block padding as possible; the
    remainder lands in the last admissible segment."""
    total = int(counts.sum())
    # grid target: fill each seg to the largest multiple of P that the
    # average load reaches (at least P)
    target = max(P, (total // nseg) // P * P)
    loads = np.zeros(nseg, np.int64)
    take = []
    for i, n in enumerate(counts):
        n = int(n)
        segs = adm[i]
        tk = {}
        for s in segs[:-1]:
            a = int(np.clip(target - loads[s], 0, n))
            tk[s] = a
            loads[s] += a
            n -= a
        tk[segs[-1]] = n
        loads[segs[-1]] += n
        take.append(tk)
    return take


def build_layer_schedule(cfg, src_by_core, dst_local_by_core, deginv_by_core,
                         table_rows, nseg=NSEG):
    """Compute the SPMD-static schedule + per-core slot arrays for one layer.

    src_by_core[c]: global table row per edge (int64)
    dst_local_by_core[c]: local node id (0..NPC-1) per edge
    deginv_by_core[c]: f32 scale per edge
    Returns (sched, percore) where sched is shared and percore is a list of
    dicts with device-ready arrays.
    """
    bases = cfg.seg_bases(table_rows, nseg)
    TPC = cfg.TPC
    bounds, adm = _intervals(bases, table_rows)
    nivl = len(adm)

    # per (core, tile, seg) edge index lists
    per_cts = [[[None] * nseg for _ in range(TPC)] for _ in range(NCORES)]
    for c in range(NCORES):
        src = src_by_core[c]
        dstl = dst_local_by_core[c]
        tile = (dstl // P).astype(np.int64)
        iv = np.searchsorted(bounds, src, side="right") - 1
        order = np.lexsort((iv, tile))
        # counts matrix [TPC, nivl]
        cnt = np.zeros((TPC, nivl), np.int64)
        np.add.at(cnt, (tile, iv), 1)
        # prefix offsets of each (t, iv) run in `order`
        run_off = np.zeros((TPC, nivl), np.int64)
        flat = cnt.reshape(-1).cumsum()
        run_off.reshape(-1)[1:] = flat[:-1]
        for t in range(TPC):
            take = _waterfill_counts(cnt[t], adm, nseg)
            buckets = {s: [] for s in range(nseg)}
            for i in range(nivl):
                o = int(run_off[t, i])
                n = int(cnt[t, i])
                pos = 0
                for s, k in take[i].items():
                    if k:
                        buckets[s].append(order[o + pos:o + pos + k])
                        pos += k
            for s in range(nseg):
                per_cts[c][t][s] = (np.concatenate(buckets[s])
                                    if buckets[s] else np.empty(0, np.int64))

    # static quotas (blocks) per (tile, seg): max over cores
    quota = np.zeros((TPC, nseg), np.int64)
    for t in range(TPC):
        for s in range(nseg):
            mx = max(len(per_cts[c][t][s]) for c in range(NCORES))
            quota[t, s] = -(-mx // P)

    # slot layout: [group][seg][tile in group][quota blocks]
    blocks = []           # list of (tile, seg) per block, in slot order
    calls = []            # list of (seg, blk_start, nblk)
    for g in range(cfg.NG):
        tiles = range(g * cfg.GSIZE, (g + 1) * cfg.GSIZE)
        for s in range(nseg):
            run_start = len(blocks)
            for t in tiles:
                for _ in range(quota[t, s]):
                    blocks.append((t, s))
            run_len = len(blocks) - run_start
            b0 = run_start
            while run_len > 0:
                nb = min(run_len, CALL_BLKS)
                calls.append((s, b0, nb))
                b0 += nb
                run_len -= nb
    totblk = len(blocks)
    sched = dict(bases=bases, quota=quota, blocks=blocks, calls=calls,
                 totblk=totblk)

    # per-core arrays
    percore = []
    for c in range(NCORES):
        idx_rel = np.zeros(totblk * P, np.int64)
        dstoff = np.zeros(totblk * P, np.float32)
        scale = np.zeros(totblk * P, np.float32)
        # fill slots tile/seg cell by cell
        cell_cursor = {}
        b_of_cell = {}
        pos = 0
        for bi, (t, s) in enumerate(blocks):
            b_of_cell.setdefault((t, s), []).append(bi)
        for (t, s), bl in b_of_cell.items():
            e = per_cts[c][t][s]
            slots = np.concatenate([np.arange(b * P, (b + 1) * P) for b in bl])
            k = len(e)
            assert k <= len(slots)
            src = src_by_core[c][e]
            idx_rel[slots[:k]] = src - bases[s]
            dstoff[slots[:k]] = (dst_local_by_core[c][e] % P).astype(np.float32)
            scale[slots[:k]] = deginv_by_core[c][e]
            # padding slots keep idx_rel 0 (valid row), scale 0
        assert idx_rel.min() >= 0 and idx_rel.max() < SEG_SPAN

        # [128, totblk] layouts (slot -> p = slot%128, k = slot//128)
        dstoff_t = dstoff.reshape(totblk, P).T.copy()
        scale_t = scale.reshape(totblk, P).T.copy()

        # int16 idx array packed per call: for call (s, b0, nb):
        # within-call j -> [j%16, col j//16], replicated over 8 groups,
        # stored at columns [b0*8, (b0+nb)*8)
        idx16 = np.zeros((P, totblk * 8), np.int16)
        for (s, b0, nb) in calls:
            n = nb * P
            j = np.arange(n)
            vals = idx_rel[b0 * P: b0 * P + n].astype(np.int16)
            cols = b0 * 8 + j // 16
            for grp in range(8):
                idx16[grp * 16 + (j % 16), cols] = vals
        percore.append(dict(idx16=idx16, dstoff=dstoff_t, scale=scale_t))
    return sched, percore


# ------------------------------------------------------------- bass build

def emit_gather_segsum(nc, tc, pools, cfg, sched, elem, tbl, idx_res, doff_res,
                       scal_res, psum_pool, psum_shape, evict_fn, qstate,
                       iota_t, gdt=F32, post_call_hooks=None):
    """Emit one layer's gather + scaled one-hot segment-sum.

    evict_fn(t, psum_tile) is called once per tile after its accumulation
    completes.
    """
    gpool, spool = pools
    blocks, calls, quota = sched["blocks"], sched["calls"], sched["quota"]
    bases = sched["bases"]
    # first/last block index per tile
    first_blk, last_blk = {}, {}
    for bi, (t, s) in enumerate(blocks):
        first_blk.setdefault(t, bi)
        last_blk[t] = bi
    psum_tiles = {}

    for ci, (s, b0, nb) in enumerate(calls):
        n_idx = nb * P
        G = gpool.tile([P, CALL_BLKS * elem], gdt, tag="G")
        lo = bases[s]
        hi = min(lo + SEG_SPAN, tbl.shape[0])
        nc.gpsimd.dma_gather(
            G[:].rearrange("p (k d) -> p k d", d=elem)[:, :nb, :],
            tbl[lo:hi, :],
            idx_res[:, b0 * 8:(b0 + nb) * 8],
            n_idx, n_idx, elem,
            single_packet=False,
            queue_num=qstate[0],
        )
        qstate[0] = (qstate[0] + 1) % 4
        # per-edge scale (also zeroes padding slots); bf16 for the PE
        GS = gpool.tile([P, CALL_BLKS * elem], BF16, tag="GS")
        G3 = G[:].rearrange("p (k d) -> p k d", d=elem)[:, :nb, :]
        GS3 = GS[:].rearrange("p (k d) -> p k d", d=elem)[:, :nb, :]
        nc.vector.tensor_tensor(
            out=GS3, in0=G3,
            in1=doff_scale_bcast(scal_res, b0, nb, elem),
            op=mybir.AluOpType.mult,
        )
        # one-hot chunks + matmuls
        for sc0 in range(0, nb, SCHUNK):
            scn = min(SCHUNK, nb - sc0)
            S = spool.tile([P, SCHUNK * P], BF16, tag="S")
            S3 = S[:].rearrange("p (k n) -> p k n", n=P)[:, :scn, :]
            nc.vector.tensor_tensor(
                out=S3,
                in0=doff_scale_bcast(doff_res, b0 + sc0, scn, P),
                in1=iota_t[:].rearrange("p (a n) -> p a n", a=1)
                    .to_broadcast([P, scn, P]),
                op=mybir.AluOpType.is_equal,
            )
            for k in range(scn):
                bi = b0 + sc0 + k
                t, _s = blocks[bi]
                if t not in psum_tiles:
                    seg_psum = psum_pool.tile(list(psum_shape), F32, tag="seg")
                    psum_tiles[t] = seg_psum
                nc.tensor.matmul(
                    psum_tiles[t][:],
                    lhsT=GS[:, (sc0 + k) * elem:(sc0 + k + 1) * elem],
                    rhs=S[:, k * P:(k + 1) * P],
                    start=(bi == first_blk[t]),
                    stop=(bi == last_blk[t]),
                )
                if bi == last_blk[t]:
                    evict_fn(t, psum_tiles.pop(t))
        if post_call_hooks and ci in post_call_hooks:
            post_call_hooks[ci]()


def doff_scale_bcast(res, k0, nb, width):
    return res[:, k0:k0 + nb].to_broadcast([P, nb, width])


def build_bass(cfg, sched1, sched2, schedL, compile_=True):
    nc = bacc.Bacc("TRN2", target_bir_lowering=False, debug=False,
                   num_devices=NCORES, num_swdge_queues=4)
    IN, HID, LAT = cfg.IN, cfg.HID, cfg.LAT
    NPCP, TPC = cfg.NPCP, cfg.TPC

    x_tbl = nc.declare_dram_parameter("x_tbl", [cfg.XROWS, IN], BF16, isOutput=False)
    xT_c = nc.declare_dram_parameter("xT_c", [IN, NPCP], F32, isOutput=False)
    idx1 = nc.declare_dram_parameter("idx1", [P, sched1["totblk"] * 8], I16, isOutput=False)
    doff1 = nc.declare_dram_parameter("doff1", [P, sched1["totblk"]], BF16, isOutput=False)
    scal1 = nc.declare_dram_parameter("scal1", [P, sched1["totblk"]], BF16, isOutput=False)
    idx2 = nc.declare_dram_parameter("idx2", [P, sched2["totblk"] * 8], I16, isOutput=False)
    idxL = nc.declare_dram_parameter("idxL", [P, schedL["totblk"] * 8], I16, isOutput=False)
    doffL = nc.declare_dram_parameter("doffL", [P, schedL["totblk"]], BF16, isOutput=False)
    scalL = nc.declare_dram_parameter("scalL", [P, schedL["totblk"]], BF16, isOutput=False)
    doff2 = nc.declare_dram_parameter("doff2", [P, sched2["totblk"]], BF16, isOutput=False)
    scal2 = nc.declare_dram_parameter("scal2", [P, sched2["totblk"]], BF16, isOutput=False)
    w1lt = nc.declare_dram_parameter("w1lt", [IN, HID], F32, isOutput=False)
    w1rt = nc.declare_dram_parameter("w1rt", [IN, HID], F32, isOutput=False)
    b1 = nc.declare_dram_parameter("b1", [HID, 1], F32, isOutput=False)
    w2lt = nc.declare_dram_parameter("w2lt", [HID, LAT], F32, isOutput=False)
    w2rt = nc.declare_dram_parameter("w2rt", [HID, LAT], F32, isOutput=False)
    b2 = nc.declare_dram_parameter("b2", [LAT, 1], F32, isOutput=False)
    wdt = nc.declare_dram_parameter("wdt", [LAT, IN], F32, isOutput=False)
    bd = nc.declare_dram_parameter("bd", [IN, 1], F32, isOutput=False)
    iota_in = nc.declare_dram_parameter("iota_in", [P, P], BF16, isOutput=False)

    zT_out = nc.declare_dram_parameter("zT_out", [LAT, NPCP], F32, isOutput=True)
    xrT_out = nc.declare_dram_parameter("xrT_out", [IN, NPCP], F32, isOutput=True)

    p_self = nc.dram_tensor("p_self", [NPCP, LAT], F32)
    msg2loc = nc.dram_tensor("msg2loc", [LAT, NPCP], F32)
    p_full = nc.dram_tensor("p_full", [cfg.PROWS, LAT], F32, addr_space="Shared")
    s2_dram = nc.dram_tensor("s2_dram", [LAT, NPCP], F32)

    with TileContext(nc) as tc:
        with tc.tile_pool(name="const", bufs=1) as cpool, \
             tc.tile_pool(name="gpool", bufs=4) as gpool, \
             tc.tile_pool(name="spool", bufs=4) as spool, \
             tc.tile_pool(name="meta", bufs=1) as mpool, \
             tc.tile_pool(name="stage", bufs=3) as stpool, \
             tc.tile_pool(name="psum", bufs=8, space="PSUM") as psum_pool:

            nc.gpsimd.load_library(mlp)

            # constants
            iota_t = cpool.tile([P, P], BF16)
            nc.sync.dma_start(out=iota_t[:], in_=iota_in[:, :])
            w1lt_t = cpool.tile([IN, HID], F32)
            nc.sync.dma_start(out=w1lt_t[:], in_=w1lt[:, :])
            w1rt_t = cpool.tile([IN, HID], F32)
            nc.sync.dma_start(out=w1rt_t[:], in_=w1rt[:, :])
            b1_t = cpool.tile([HID, 1], F32)
            nc.sync.dma_start(out=b1_t[:], in_=b1[:, :])
            w2lt_t = cpool.tile([HID, LAT], F32)
            nc.sync.dma_start(out=w2lt_t[:], in_=w2lt[:, :])
            w2rt_t = cpool.tile([HID, LAT], F32)
            nc.sync.dma_start(out=w2rt_t[:], in_=w2rt[:, :])
            b2_t = cpool.tile([LAT, 1], F32)
            nc.sync.dma_start(out=b2_t[:], in_=b2[:, :])
            wdt_t = cpool.tile([LAT, IN], F32)
            nc.sync.dma_start(out=wdt_t[:], in_=wdt[:, :])
            bd_t = cpool.tile([IN, 1], F32)
            nc.sync.dma_start(out=bd_t[:], in_=bd[:, :])

            # resident metadata, layer 1
            idx1_t = mpool.tile([P, sched1["totblk"] * 8], I16, tag="idx")
            nc.sync.dma_start(out=idx1_t[:], in_=idx1[:, :])
            doff1_t = mpool.tile([P, sched1["totblk"]], BF16)
            nc.sync.dma_start(out=doff1_t[:], in_=doff1[:, :])
            scal1_t = mpool.tile([P, sched1["totblk"]], BF16)
            nc.sync.dma_start(out=scal1_t[:], in_=scal1[:, :])

            qstate = [0]

            def evict1(t, psum_t):
                c0, c1 = t * P, (t + 1) * P
                aggT = stpool.tile([IN, P], F32, tag="aggT")
                nc.vector.tensor_copy(out=aggT[:], in_=psum_t[:])
                xT_t = stpool.tile([IN, P], F32, tag="xTt")
                nc.sync.dma_start(out=xT_t[:], in_=xT_c[:, c0:c1])
                hps = psum_pool.tile([HID, P], F32, tag="seg")
                nc.tensor.matmul(hps[:], lhsT=w1lt_t[:], rhs=aggT[:],
                                 start=True, stop=False)
                nc.tensor.matmul(hps[:], lhsT=w1rt_t[:], rhs=xT_t[:],
                                 start=False, stop=True)
                hT = stpool.tile([HID, P], F32, tag="hT")
                nc.scalar.activation(hT[:], hps[:],
                                     mybir.ActivationFunctionType.Relu,
                                     bias=b1_t[:])
                pns = psum_pool.tile([P, LAT], F32, tag="seg")
                nc.tensor.matmul(pns[:], lhsT=hT[:], rhs=w2lt_t[:],
                                 start=True, stop=True)
                s2s = psum_pool.tile([LAT, P], F32, tag="seg")
                nc.tensor.matmul(s2s[:], lhsT=w2rt_t[:], rhs=hT[:],
                                 start=True, stop=True)
                pn = stpool.tile([P, LAT], F32, tag="pn")
                nc.vector.tensor_copy(out=pn[:], in_=pns[:])
                s2 = stpool.tile([LAT, P], F32, tag="s2")
                nc.vector.tensor_scalar_add(s2[:], s2s[:], b2_t[:])
                nc.sync.dma_start(out=p_self[c0:c1, :], in_=pn[:])
                nc.sync.dma_start(out=s2_dram[:, c0:c1], in_=s2[:])

            emit_gather_segsum(nc, tc, (gpool, spool), cfg, sched1, IN,
                               x_tbl, idx1_t[:], doff1_t[:], scal1_t[:],
                               psum_pool, (IN, P), evict1, qstate, iota_t,
                               gdt=BF16)

            # AllGather p (trigger waits on p_self, then CC runs in
            # parallel with the local-edge gather phase below)
            nc.gpsimd.collective_compute(
                "AllGather", mybir.AluOpType.bypass,
                replica_groups=[list(range(NCORES))],
                ins=[p_self.ap().opt()],
                outs=[p_full.ap().opt()],
            )

            # local-edge gather phase: own-core sources from p_self, no AG dep
            idxL_t = mpool.tile([P, schedL["totblk"] * 8], I16)
            nc.sync.dma_start(out=idxL_t[:], in_=idxL[:, :])
            doffL_t = mpool.tile([P, schedL["totblk"]], BF16)
            nc.sync.dma_start(out=doffL_t[:], in_=doffL[:, :])
            scalL_t = mpool.tile([P, schedL["totblk"]], BF16)
            nc.sync.dma_start(out=scalL_t[:], in_=scalL[:, :])

            def evict_loc(t, psum_t):
                c0, c1 = t * P, (t + 1) * P
                locT = stpool.tile([LAT, P], F32, tag="locT")
                nc.vector.tensor_copy(out=locT[:], in_=psum_t[:])
                nc.sync.dma_start(out=msg2loc[:, c0:c1], in_=locT[:])

            emit_gather_segsum(nc, tc, (gpool, spool), cfg, schedL, LAT,
                               p_self, idxL_t[:], doffL_t[:], scalL_t[:],
                               psum_pool, (LAT, P), evict_loc, qstate, iota_t)

            # resident metadata, layer 2
            idx2_t = mpool.tile([P, sched2["totblk"] * 8], I16, tag="idx")
            nc.sync.dma_start(out=idx2_t[:], in_=idx2[:, :])
            doff2_t = mpool.tile([P, sched2["totblk"]], BF16)
            nc.sync.dma_start(out=doff2_t[:], in_=doff2[:, :])
            scal2_t = mpool.tile([P, sched2["totblk"]], BF16)
            nc.sync.dma_start(out=scal2_t[:], in_=scal2[:, :])

            def evict2(t, psum_t):
                c0, c1 = t * P, (t + 1) * P
                s2t = stpool.tile([LAT, P], F32, tag="s2t")
                nc.sync.dma_start(out=s2t[:], in_=s2_dram[:, c0:c1])
                loct = stpool.tile([LAT, P], F32, tag="loct")
                nc.sync.dma_start(out=loct[:], in_=msg2loc[:, c0:c1])
                zT = stpool.tile([LAT, P], F32, tag="zT")
                nc.vector.tensor_add(out=zT[:], in0=psum_t[:], in1=s2t[:])
                nc.vector.tensor_add(out=zT[:], in0=zT[:], in1=loct[:])
                xrp = psum_pool.tile([IN, P], F32, tag="seg")
                nc.tensor.matmul(xrp[:], lhsT=wdt_t[:], rhs=zT[:],
                                 start=True, stop=True)
                xr = stpool.tile([IN, P], F32, tag="xr")
                nc.vector.tensor_scalar_add(xr[:], xrp[:], bd_t[:])
                nc.sync.dma_start(out=zT_out[:, c0:c1], in_=zT[:])
                nc.sync.dma_start(out=xrT_out[:, c0:c1], in_=xr[:])

            emit_gather_segsum(nc, tc, (gpool, spool), cfg, sched2, LAT,
                               p_full, idx2_t[:], doff2_t[:], scal2_t[:],
                               psum_pool, (LAT, P), evict2, qstate, iota_t)

    if compile_:
        nc.compile()
    return nc


# ------------------------------------------------------------ host driver

def prepare(x, edge_index, W1_l, b1_, W1_r, W2_l, b2_, W2_r, W_dec, b_dec,
            cfg):
    src = np.asarray(edge_index[0], np.int64)
    dst = np.asarray(edge_index[1], np.int64)
    N, E = cfg.N, cfg.E
    deg = np.bincount(dst, minlength=N).astype(np.float32)
    deginv = 1.0 / np.maximum(deg, 1.0)

    owner = dst // cfg.NPC
    src_by, dstl_by, dgi_by = [], [], []
    for c in range(NCORES):
        m = np.nonzero(owner == c)[0]
        src_by.append(src[m])
        dstl_by.append(dst[m] - c * cfg.NPC)
        dgi_by.append(deginv[dst[m]])

    sched1, pc1 = build_layer_schedule(cfg, src_by, dstl_by, dgi_by, cfg.XROWS)

    # layer-2 local/remote split: edges whose src this core owns can gather
    # from p_self before the AllGather completes.  Per-tile local quotas come
    # from the cross-core min (rounded to the block grid) to minimize padding.
    TPC = cfg.TPC
    locm_by = [(src_by[c] // cfg.NPC) == c for c in range(NCORES)]
    tile_by = [dstl_by[c] // P for c in range(NCORES)]
    loc_cnt = np.zeros((NCORES, TPC), np.int64)
    for c in range(NCORES):
        np.add.at(loc_cnt[c], tile_by[c][locm_by[c]], 1)
    qloc = np.maximum(P, (np.rint(loc_cnt.min(axis=0) / P) * P)).astype(np.int64)
    srcL_by, dstlL_by, dgiL_by = [], [], []
    src2_by, dstlR_by, dgiR_by = [], [], []
    for c in range(NCORES):
        sel = np.zeros(len(src_by[c]), bool)
        for t in range(TPC):
            e = np.nonzero(locm_by[c] & (tile_by[c] == t))[0]
            take = min(len(e), int(qloc[t]))
            sel[e[:take]] = True
        srcL_by.append(src_by[c][sel] - c * cfg.NPC)
        dstlL_by.append(dstl_by[c][sel])
        dgiL_by.append(dgi_by[c][sel])
        r = ~sel
        so = src_by[c][r] // cfg.NPC
        src2_by.append(so * cfg.NPCP + (src_by[c][r] - so * cfg.NPC))
        dstlR_by.append(dstl_by[c][r])
        dgiR_by.append(dgi_by[c][r])

    schedL, pcL = build_layer_schedule(cfg, srcL_by, dstlL_by, dgiL_by,
                                       cfg.NPCP, nseg=1)
    sched2, pc2 = build_layer_schedule(cfg, src2_by, dstlR_by, dgiR_by,
                                       cfg.PROWS)

    iota = np.broadcast_to(np.arange(P), (P, P)).astype(BF)
    shared = dict(
        x_tbl=np.ascontiguousarray(np.asarray(x, np.float32).astype(BF)),
        w1lt=np.ascontiguousarray(W1_l.T, np.float32),
        w1rt=np.ascontiguousarray(W1_r.T, np.float32),
        b1=np.ascontiguousarray(b1_.reshape(-1, 1), np.float32),
        w2lt=np.ascontiguousarray(W2_l.T, np.float32),
        w2rt=np.ascontiguousarray(W2_r.T, np.float32),
        b2=np.ascontiguousarray(b2_.reshape(-1, 1), np.float32),
        wdt=np.ascontiguousarray(W_dec.T, np.float32),
        bd=np.ascontiguousarray(b_dec.reshape(-1, 1), np.float32),
        iota_in=iota,
    )
    in_maps = []
    for c in range(NCORES):
        xs = np.zeros((cfg.IN, cfg.NPCP), np.float32)
        xs[:, :cfg.NPC] = x[c * cfg.NPC:(c + 1) * cfg.NPC].T
        m = dict(shared)
        m.update(
            xT_c=xs,
            idx1=pc1[c]["idx16"], doff1=pc1[c]["dstoff"].astype(BF),
            scal1=pc1[c]["scale"].astype(BF),
            idx2=pc2[c]["idx16"], doff2=pc2[c]["dstoff"].astype(BF),
            scal2=pc2[c]["scale"].astype(BF),
            idxL=pcL[c]["idx16"], doffL=pcL[c]["dstoff"].astype(BF),
            scalL=pcL[c]["scale"].astype(BF),
        )
        in_maps.append(m)
    return sched1, sched2, schedL, in_maps


def assemble(results, cfg):
    xr = np.empty((cfg.N, cfg.IN), np.float32)
    z = np.empty((cfg.N, cfg.LAT), np.float32)
    for c in range(NCORES):
        xr[c * cfg.NPC:(c + 1) * cfg.NPC] = results[c]["xrT_out"][:, :cfg.NPC].T
        z[c * cfg.NPC:(c + 1) * cfg.NPC] = results[c]["zT_out"][:, :cfg.NPC].T
    return xr, z


def kernel(x, edge_index, W1_l, b1_l, W1_r, W2_l, b2_l, W2_r, W_dec, b_dec,
           _trace=False):
    x = np.asarray(x)
    edge_index = np.asarray(edge_index)
    n, e = x.shape[0], edge_index.shape[1]
    cfg = Cfg(n, e, x.shape[1], W1_l.shape[0], W2_l.shape[0],
              n // NCORES, gsize=7 if n == 100000 else 2)
    sched1, sched2, schedL, in_maps = prepare(
        x, edge_index, np.asarray(W1_l), np.asarray(b1_l), np.asarray(W1_r),
        np.asarray(W2_l), np.asarray(b2_l), np.asarray(W2_r),
        np.asarray(W_dec), np.asarray(b_dec), cfg)

    nc = build_bass(cfg, sched1, sched2, schedL)
    res = run_bass_kernel_spmd(nc, in_maps, core_ids=list(range(NCORES)),
                               trace=_trace)
    xr, z = assemble(res.results, cfg)
    kernel.last_exec_ns = res.exec_time_ns
    return (xr, z)


# revision 25
# speedup vs baseline: 1.1065x; 1.0870x over previous
"""Distributed Trainium2 (8 NeuronCores) kernel for a 2-layer GraphSAGE
autoencoder (mean aggregation) — nn_AnomalyDetector.

Strategy (vertex-cut by destination owner):
  - Nodes are sharded contiguously across 8 cores (12500 each, padded to
    12544 = 98 tiles x 128 slots).  Each core owns the edges whose dst it
    owns; segment-sum is local.
  - Layer 1 gathers rows of the (replicated) x table with dma_gather
    (int16 indices -> 4 overlapping 32768-row table segments; per-edge
    segment assignment is waterfilled per destination tile, with per-run
    static block quotas shared by all 8 cores so the SPMD graph is
    identical).
  - Segment-sum is a PE matmul against a one-hot matrix built on DVE
    (is_equal vs an iota row), accumulated in PSUM per destination tile.
    Per-edge 1/deg scaling is folded into the gathered rows, so padding
    slots (scale 0) contribute nothing.
  - Feat-major layouts throughout: aggT [f,n] -> hT [h,n] -> p [n,l]
    (node-major via operand swap) + s2T [l,n].  p is AllGathered to the
    full [100352, 64] table for the layer-2 gather.  Outputs xrecT
    [128, 12544] and zT [64, 12544] are transposed/trimmed on the host.
"""

import os
import sys

for _p in ("/opt/trn_rl_repo", "/root/.axon_site/_ro/trn_rl_repo"):
    if os.path.isdir(_p) and _p not in sys.path:
        sys.path.append(_p)

import numpy as np
import ml_dtypes

BF = ml_dtypes.bfloat16

import concourse.bass as bass
import concourse.mybir as mybir
from concourse import bacc
from concourse.tile import TileContext
from concourse.bass_utils import run_bass_kernel_spmd
from concourse.library_config import mlp

F32 = mybir.dt.float32
BF16 = mybir.dt.bfloat16
I16 = mybir.dt.int16

NCORES = 8
P = 128          # partitions / block size / tile node count
SEG_SPAN = 32768  # int16 index reach
NSEG = 4
SCHUNK = 16      # blocks per one-hot build
CALL_BLKS = 64   # max blocks per dma_gather call


# ----------------------------------------------------------------- config

class Cfg:
    def __init__(self, n, e, in_ch, hid_ch, lat_ch, npc, gsize):
        assert n % NCORES == 0 or True
        self.N, self.E = n, e
        self.IN, self.HID, self.LAT = in_ch, hid_ch, lat_ch
        self.NPC = npc                        # real nodes per core
        self.NPCP = -(-npc // P) * P          # padded
        self.TPC = self.NPCP // P             # tiles per core
        self.GSIZE = gsize                    # tiles per group
        assert self.TPC % gsize == 0
        self.NG = self.TPC // gsize
        # table sizes
        self.XROWS = n                        # layer-1 table rows
        self.PROWS = self.NPCP * NCORES       # layer-2 table rows

    def seg_bases(self, rows, nseg=NSEG):
        if nseg == 1 or rows <= SEG_SPAN:
            return [0] * nseg
        step = -(-(rows - SEG_SPAN) // (nseg - 1))
        return [min(s * step, rows - SEG_SPAN) for s in range(nseg)]


# ---------------------------------------------------------- host schedule

def _intervals(bases, rows):
    """Split [0, rows) into intervals with constant admissible-segment sets.
    Returns (bounds, adm) where bounds has len nivl+1 and adm[i] is the
    ordered list of admissible segs for interval i."""
    nseg = len(bases)
    lo = np.array(bases)
    hi = np.minimum(lo + SEG_SPAN, rows)
    cuts = np.unique(np.concatenate([lo, hi, [0, rows]]))
    cuts = cuts[(cuts >= 0) & (cuts <= rows)]
    adm = []
    bounds = []
    for i in range(len(cuts) - 1):
        a, b = cuts[i], cuts[i + 1]
        if a == b:
            continue
        mid = (a + b) // 2
        segs = [s for s in range(nseg) if lo[s] <= mid < hi[s]]
        assert segs, f"uncovered interval [{a},{b})"
        bounds.append(a)
        adm.append(segs)
    bounds.append(rows)
    return np.array(bounds), adm


def _waterfill_counts(counts, adm, nseg=NSEG):
    """counts[i] edges in interval i with admissible segs adm[i].
    Returns take[i][s] = how many of interval i go to seg s.

    Fills each non-final segment up to a multiple of P (block grid) so the
    per-(tile,seg) quotas waste as little block padding as possible; the
    remainder lands in the last admissible segment."""
    total = int(counts.sum())
    # baseline grid target for segments with no mandatory load yet
    gtarget = max(P, (total // nseg) // P * P)
    loads = np.zeros(nseg, np.int64)
    take = []
    for i, n in enumerate(counts):
        n = int(n)
        segs = adm[i]
        tk = {}
        for s in segs[:-1]:
            # top this segment up to its own next block-grid point (or the
            # baseline target) so its quota has no padding; spill the rest
            tgt = max(-(-int(loads[s]) // P) * P, gtarget)
            a = int(np.clip(tgt - loads[s], 0, n))
            tk[s] = a
            loads[s] += a
            n -= a
        tk[segs[-1]] = n
        loads[segs[-1]] += n
        take.append(tk)
    return take


def build_layer_schedule(cfg, src_by_core, dst_local_by_core, deginv_by_core,
                         table_rows, nseg=NSEG):
    """Compute the SPMD-static schedule + per-core slot arrays for one layer.

    src_by_core[c]: global table row per edge (int64)
    dst_local_by_core[c]: local node id (0..NPC-1) per edge
    deginv_by_core[c]: f32 scale per edge
    Returns (sched, percore) where sched is shared and percore is a list of
    dicts with device-ready arrays.
    """
    bases = cfg.seg_bases(table_rows, nseg)
    TPC = cfg.TPC
    bounds, adm = _intervals(bases, table_rows)
    nivl = len(adm)

    # per (core, tile, seg) edge index lists
    per_cts = [[[None] * nseg for _ in range(TPC)] for _ in range(NCORES)]
    for c in range(NCORES):
        src = src_by_core[c]
        dstl = dst_local_by_core[c]
        tile = (dstl // P).astype(np.int64)
        iv = np.searchsorted(bounds, src, side="right") - 1
        order = np.lexsort((iv, tile))
        # counts matrix [TPC, nivl]
        cnt = np.zeros((TPC, nivl), np.int64)
        np.add.at(cnt, (tile, iv), 1)
        # prefix offsets of each (t, iv) run in `order`
        run_off = np.zeros((TPC, nivl), np.int64)
        flat = cnt.reshape(-1).cumsum()
        run_off.reshape(-1)[1:] = flat[:-1]
        for t in range(TPC):
            take = _waterfill_counts(cnt[t], adm, nseg)
            buckets = {s: [] for s in range(nseg)}
            for i in range(nivl):
                o = int(run_off[t, i])
                n = int(cnt[t, i])
                pos = 0
                for s, k in take[i].items():
                    if k:
                        buckets[s].append(order[o + pos:o + pos + k])
                        pos += k
            for s in range(nseg):
                per_cts[c][t][s] = (np.concatenate(buckets[s])
                                    if buckets[s] else np.empty(0, np.int64))

    # static quotas (blocks) per (tile, seg): max over cores
    quota = np.zeros((TPC, nseg), np.int64)
    for t in range(TPC):
        for s in range(nseg):
            mx = max(len(per_cts[c][t][s]) for c in range(NCORES))
            quota[t, s] = -(-mx // P)

    # slot layout: [group][seg][tile in group][quota blocks]
    blocks = []           # list of (tile, seg) per block, in slot order
    calls = []            # list of (seg, blk_start, nblk)
    for g in range(cfg.NG):
        tiles = range(g * cfg.GSIZE, (g + 1) * cfg.GSIZE)
        for s in range(nseg):
            run_start = len(blocks)
            for t in tiles:
                for _ in range(quota[t, s]):
                    blocks.append((t, s))
            run_len = len(blocks) - run_start
            b0 = run_start
            while run_len > 0:
                nb = min(run_len, CALL_BLKS)
                calls.append((s, b0, nb))
                b0 += nb
                run_len -= nb
    totblk = len(blocks)
    sched = dict(bases=bases, quota=quota, blocks=blocks, calls=calls,
                 totblk=totblk)

    # per-core arrays
    percore = []
    for c in range(NCORES):
        idx_rel = np.zeros(totblk * P, np.int64)
        dstoff = np.zeros(totblk * P, np.float32)
        scale = np.zeros(totblk * P, np.float32)
        # fill slots tile/seg cell by cell
        cell_cursor = {}
        b_of_cell = {}
        pos = 0
        for bi, (t, s) in enumerate(blocks):
            b_of_cell.setdefault((t, s), []).append(bi)
        for (t, s), bl in b_of_cell.items():
            e = per_cts[c][t][s]
            slots = np.concatenate([np.arange(b * P, (b + 1) * P) for b in bl])
            k = len(e)
            assert k <= len(slots)
            src = src_by_core[c][e]
            idx_rel[slots[:k]] = src - bases[s]
            dstoff[slots[:k]] = (dst_local_by_core[c][e] % P).astype(np.float32)
            scale[slots[:k]] = deginv_by_core[c][e]
            # padding slots keep idx_rel 0 (valid row), scale 0
        assert idx_rel.min() >= 0 and idx_rel.max() < SEG_SPAN

        # [128, totblk] layouts (slot -> p = slot%128, k = slot//128)
        dstoff_t = dstoff.reshape(totblk, P).T.copy()
        scale_t = scale.reshape(totblk, P).T.copy()

        # int16 idx array packed per call: for call (s, b0, nb):
        # within-call j -> [j%16, col j//16], replicated over 8 groups,
        # stored at columns [b0*8, (b0+nb)*8)
        idx16 = np.zeros((P, totblk * 8), np.int16)
        for (s, b0, nb) in calls:
            n = nb * P
            j = np.arange(n)
            vals = idx_rel[b0 * P: b0 * P + n].astype(np.int16)
            cols = b0 * 8 + j // 16
            for grp in range(8):
                idx16[grp * 16 + (j % 16), cols] = vals
        percore.append(dict(idx16=idx16, dstoff=dstoff_t, scale=scale_t))
    return sched, percore


# ------------------------------------------------------------- bass build

def emit_gather_segsum(nc, tc, pools, cfg, sched, elem, tbl, idx_res, doff_res,
                       scal_res, psum_pool, psum_shape, evict_fn, qstate,
                       iota_t, gdt=F32, post_call_hooks=None):
    """Emit one layer's gather + scaled one-hot segment-sum.

    evict_fn(t, psum_tile) is called once per tile after its accumulation
    completes.
    """
    gpool, spool = pools
    blocks, calls, quota = sched["blocks"], sched["calls"], sched["quota"]
    bases = sched["bases"]
    # first/last block index per tile
    first_blk, last_blk = {}, {}
    for bi, (t, s) in enumerate(blocks):
        first_blk.setdefault(t, bi)
        last_blk[t] = bi
    psum_tiles = {}

    for ci, (s, b0, nb) in enumerate(calls):
        n_idx = nb * P
        G = gpool.tile([P, CALL_BLKS * elem], gdt, tag="G")
        lo = bases[s]
        hi = min(lo + SEG_SPAN, tbl.shape[0])
        nc.gpsimd.dma_gather(
            G[:].rearrange("p (k d) -> p k d", d=elem)[:, :nb, :],
            tbl[lo:hi, :],
            idx_res[:, b0 * 8:(b0 + nb) * 8],
            n_idx, n_idx, elem,
            single_packet=False,
            queue_num=qstate[0],
        )
        qstate[0] = (qstate[0] + 1) % 4
        # per-edge scale (also zeroes padding slots); bf16 for the PE
        GS = gpool.tile([P, CALL_BLKS * elem], BF16, tag="GS")
        G3 = G[:].rearrange("p (k d) -> p k d", d=elem)[:, :nb, :]
        GS3 = GS[:].rearrange("p (k d) -> p k d", d=elem)[:, :nb, :]
        nc.vector.tensor_tensor(
            out=GS3, in0=G3,
            in1=doff_scale_bcast(scal_res, b0, nb, elem),
            op=mybir.AluOpType.mult,
        )
        # one-hot chunks + matmuls
        for sc0 in range(0, nb, SCHUNK):
            scn = min(SCHUNK, nb - sc0)
            S = spool.tile([P, SCHUNK * P], BF16, tag="S")
            S3 = S[:].rearrange("p (k n) -> p k n", n=P)[:, :scn, :]
            nc.vector.tensor_tensor(
                out=S3,
                in0=doff_scale_bcast(doff_res, b0 + sc0, scn, P),
                in1=iota_t[:].rearrange("p (a n) -> p a n", a=1)
                    .to_broadcast([P, scn, P]),
                op=mybir.AluOpType.is_equal,
            )
            for k in range(scn):
                bi = b0 + sc0 + k
                t, _s = blocks[bi]
                if t not in psum_tiles:
                    seg_psum = psum_pool.tile(list(psum_shape), F32, tag="seg")
                    psum_tiles[t] = seg_psum
                nc.tensor.matmul(
                    psum_tiles[t][:],
                    lhsT=GS[:, (sc0 + k) * elem:(sc0 + k + 1) * elem],
                    rhs=S[:, k * P:(k + 1) * P],
                    start=(bi == first_blk[t]),
                    stop=(bi == last_blk[t]),
                )
                if bi == last_blk[t]:
                    evict_fn(t, psum_tiles.pop(t))
        if post_call_hooks and ci in post_call_hooks:
            post_call_hooks[ci]()


def doff_scale_bcast(res, k0, nb, width):
    return res[:, k0:k0 + nb].to_broadcast([P, nb, width])


def build_bass(cfg, sched1, sched2, schedL, compile_=True):
    nc = bacc.Bacc("TRN2", target_bir_lowering=False, debug=False,
                   num_devices=NCORES, num_swdge_queues=4)
    IN, HID, LAT = cfg.IN, cfg.HID, cfg.LAT
    NPCP, TPC = cfg.NPCP, cfg.TPC

    x_tbl = nc.declare_dram_parameter("x_tbl", [cfg.XROWS, IN], BF16, isOutput=False)
    xT_c = nc.declare_dram_parameter("xT_c", [IN, NPCP], F32, isOutput=False)
    idx1 = nc.declare_dram_parameter("idx1", [P, sched1["totblk"] * 8], I16, isOutput=False)
    doff1 = nc.declare_dram_parameter("doff1", [P, sched1["totblk"]], BF16, isOutput=False)
    scal1 = nc.declare_dram_parameter("scal1", [P, sched1["totblk"]], BF16, isOutput=False)
    idx2 = nc.declare_dram_parameter("idx2", [P, sched2["totblk"] * 8], I16, isOutput=False)
    idxL = nc.declare_dram_parameter("idxL", [P, schedL["totblk"] * 8], I16, isOutput=False)
    doffL = nc.declare_dram_parameter("doffL", [P, schedL["totblk"]], BF16, isOutput=False)
    scalL = nc.declare_dram_parameter("scalL", [P, schedL["totblk"]], BF16, isOutput=False)
    doff2 = nc.declare_dram_parameter("doff2", [P, sched2["totblk"]], BF16, isOutput=False)
    scal2 = nc.declare_dram_parameter("scal2", [P, sched2["totblk"]], BF16, isOutput=False)
    w1lt = nc.declare_dram_parameter("w1lt", [IN, HID], F32, isOutput=False)
    w1rt = nc.declare_dram_parameter("w1rt", [IN, HID], F32, isOutput=False)
    b1 = nc.declare_dram_parameter("b1", [HID, 1], F32, isOutput=False)
    w2lt = nc.declare_dram_parameter("w2lt", [HID, LAT], F32, isOutput=False)
    w2rt = nc.declare_dram_parameter("w2rt", [HID, LAT], F32, isOutput=False)
    b2 = nc.declare_dram_parameter("b2", [LAT, 1], F32, isOutput=False)
    wdt = nc.declare_dram_parameter("wdt", [LAT, IN], F32, isOutput=False)
    bd = nc.declare_dram_parameter("bd", [IN, 1], F32, isOutput=False)
    iota_in = nc.declare_dram_parameter("iota_in", [P, P], BF16, isOutput=False)

    zT_out = nc.declare_dram_parameter("zT_out", [LAT, NPCP], F32, isOutput=True)
    xrT_out = nc.declare_dram_parameter("xrT_out", [IN, NPCP], F32, isOutput=True)

    p_self = nc.dram_tensor("p_self", [NPCP, LAT], F32)
    msg2loc = nc.dram_tensor("msg2loc", [LAT, NPCP], F32)
    p_full = nc.dram_tensor("p_full", [cfg.PROWS, LAT], F32, addr_space="Shared")
    s2_dram = nc.dram_tensor("s2_dram", [LAT, NPCP], F32)

    with TileContext(nc) as tc:
        with tc.tile_pool(name="const", bufs=1) as cpool, \
             tc.tile_pool(name="gpool", bufs=4) as gpool, \
             tc.tile_pool(name="spool", bufs=4) as spool, \
             tc.tile_pool(name="meta", bufs=1) as mpool, \
             tc.tile_pool(name="stage", bufs=3) as stpool, \
             tc.tile_pool(name="psum", bufs=8, space="PSUM") as psum_pool:

            nc.gpsimd.load_library(mlp)

            # constants
            iota_t = cpool.tile([P, P], BF16)
            nc.sync.dma_start(out=iota_t[:], in_=iota_in[:, :])
            w1lt_t = cpool.tile([IN, HID], F32)
            nc.sync.dma_start(out=w1lt_t[:], in_=w1lt[:, :])
            w1rt_t = cpool.tile([IN, HID], F32)
            nc.sync.dma_start(out=w1rt_t[:], in_=w1rt[:, :])
            b1_t = cpool.tile([HID, 1], F32)
            nc.sync.dma_start(out=b1_t[:], in_=b1[:, :])
            w2lt_t = cpool.tile([HID, LAT], F32)
            nc.sync.dma_start(out=w2lt_t[:], in_=w2lt[:, :])
            w2rt_t = cpool.tile([HID, LAT], F32)
            nc.sync.dma_start(out=w2rt_t[:], in_=w2rt[:, :])
            b2_t = cpool.tile([LAT, 1], F32)
            nc.sync.dma_start(out=b2_t[:], in_=b2[:, :])
            wdt_t = cpool.tile([LAT, IN], F32)
            nc.sync.dma_start(out=wdt_t[:], in_=wdt[:, :])
            bd_t = cpool.tile([IN, 1], F32)
            nc.sync.dma_start(out=bd_t[:], in_=bd[:, :])

            # resident metadata, layer 1
            idx1_t = mpool.tile([P, sched1["totblk"] * 8], I16, tag="idx")
            nc.sync.dma_start(out=idx1_t[:], in_=idx1[:, :])
            doff1_t = mpool.tile([P, sched1["totblk"]], BF16)
            nc.sync.dma_start(out=doff1_t[:], in_=doff1[:, :])
            scal1_t = mpool.tile([P, sched1["totblk"]], BF16)
            nc.sync.dma_start(out=scal1_t[:], in_=scal1[:, :])

            qstate = [0]

            def evict1(t, psum_t):
                c0, c1 = t * P, (t + 1) * P
                aggT = stpool.tile([IN, P], F32, tag="aggT")
                nc.vector.tensor_copy(out=aggT[:], in_=psum_t[:])
                xT_t = stpool.tile([IN, P], F32, tag="xTt")
                nc.sync.dma_start(out=xT_t[:], in_=xT_c[:, c0:c1])
                hps = psum_pool.tile([HID, P], F32, tag="seg")
                nc.tensor.matmul(hps[:], lhsT=w1lt_t[:], rhs=aggT[:],
                                 start=True, stop=False)
                nc.tensor.matmul(hps[:], lhsT=w1rt_t[:], rhs=xT_t[:],
                                 start=False, stop=True)
                hT = stpool.tile([HID, P], F32, tag="hT")
                nc.scalar.activation(hT[:], hps[:],
                                     mybir.ActivationFunctionType.Relu,
                                     bias=b1_t[:])
                pns = psum_pool.tile([P, LAT], F32, tag="seg")
                nc.tensor.matmul(pns[:], lhsT=hT[:], rhs=w2lt_t[:],
                                 start=True, stop=True)
                s2s = psum_pool.tile([LAT, P], F32, tag="seg")
                nc.tensor.matmul(s2s[:], lhsT=w2rt_t[:], rhs=hT[:],
                                 start=True, stop=True)
                pn = stpool.tile([P, LAT], F32, tag="pn")
                nc.vector.tensor_copy(out=pn[:], in_=pns[:])
                s2 = stpool.tile([LAT, P], F32, tag="s2")
                nc.vector.tensor_scalar_add(s2[:], s2s[:], b2_t[:])
                nc.sync.dma_start(out=p_self[c0:c1, :], in_=pn[:])
                nc.sync.dma_start(out=s2_dram[:, c0:c1], in_=s2[:])

            emit_gather_segsum(nc, tc, (gpool, spool), cfg, sched1, IN,
                               x_tbl, idx1_t[:], doff1_t[:], scal1_t[:],
                               psum_pool, (IN, P), evict1, qstate, iota_t,
                               gdt=BF16)

            # AllGather p (trigger waits on p_self, then CC runs in
            # parallel with the local-edge gather phase below)
            nc.gpsimd.collective_compute(
                "AllGather", mybir.AluOpType.bypass,
                replica_groups=[list(range(NCORES))],
                ins=[p_self.ap().opt()],
                outs=[p_full.ap().opt()],
            )

            # local-edge gather phase: own-core sources from p_self, no AG dep
            idxL_t = mpool.tile([P, schedL["totblk"] * 8], I16)
            nc.sync.dma_start(out=idxL_t[:], in_=idxL[:, :])
            doffL_t = mpool.tile([P, schedL["totblk"]], BF16)
            nc.sync.dma_start(out=doffL_t[:], in_=doffL[:, :])
            scalL_t = mpool.tile([P, schedL["totblk"]], BF16)
            nc.sync.dma_start(out=scalL_t[:], in_=scalL[:, :])

            def evict_loc(t, psum_t):
                c0, c1 = t * P, (t + 1) * P
                locT = stpool.tile([LAT, P], F32, tag="locT")
                nc.vector.tensor_copy(out=locT[:], in_=psum_t[:])
                nc.sync.dma_start(out=msg2loc[:, c0:c1], in_=locT[:])

            emit_gather_segsum(nc, tc, (gpool, spool), cfg, schedL, LAT,
                               p_self, idxL_t[:], doffL_t[:], scalL_t[:],
                               psum_pool, (LAT, P), evict_loc, qstate, iota_t)

            # resident metadata, layer 2
            idx2_t = mpool.tile([P, sched2["totblk"] * 8], I16, tag="idx")
            nc.sync.dma_start(out=idx2_t[:], in_=idx2[:, :])
            doff2_t = mpool.tile([P, sched2["totblk"]], BF16)
            nc.sync.dma_start(out=doff2_t[:], in_=doff2[:, :])
            scal2_t = mpool.tile([P, sched2["totblk"]], BF16)
            nc.sync.dma_start(out=scal2_t[:], in_=scal2[:, :])

            def evict2(t, psum_t):
                c0, c1 = t * P, (t + 1) * P
                s2t = stpool.tile([LAT, P], F32, tag="s2t")
                nc.sync.dma_start(out=s2t[:], in_=s2_dram[:, c0:c1])
                loct = stpool.tile([LAT, P], F32, tag="loct")
                nc.sync.dma_start(out=loct[:], in_=msg2loc[:, c0:c1])
                zT = stpool.tile([LAT, P], F32, tag="zT")
                nc.vector.tensor_add(out=zT[:], in0=psum_t[:], in1=s2t[:])
                nc.vector.tensor_add(out=zT[:], in0=zT[:], in1=loct[:])
                xrp = psum_pool.tile([IN, P], F32, tag="seg")
                nc.tensor.matmul(xrp[:], lhsT=wdt_t[:], rhs=zT[:],
                                 start=True, stop=True)
                xr = stpool.tile([IN, P], F32, tag="xr")
                nc.vector.tensor_scalar_add(xr[:], xrp[:], bd_t[:])
                nc.sync.dma_start(out=zT_out[:, c0:c1], in_=zT[:])
                nc.sync.dma_start(out=xrT_out[:, c0:c1], in_=xr[:])

            emit_gather_segsum(nc, tc, (gpool, spool), cfg, sched2, LAT,
                               p_full, idx2_t[:], doff2_t[:], scal2_t[:],
                               psum_pool, (LAT, P), evict2, qstate, iota_t)

    if compile_:
        nc.compile()
    return nc


# ------------------------------------------------------------ host driver

def prepare(x, edge_index, W1_l, b1_, W1_r, W2_l, b2_, W2_r, W_dec, b_dec,
            cfg):
    src = np.asarray(edge_index[0], np.int64)
    dst = np.asarray(edge_index[1], np.int64)
    N, E = cfg.N, cfg.E
    deg = np.bincount(dst, minlength=N).astype(np.float32)
    deginv = 1.0 / np.maximum(deg, 1.0)

    owner = dst // cfg.NPC
    src_by, dstl_by, dgi_by = [], [], []
    for c in range(NCORES):
        m = np.nonzero(owner == c)[0]
        src_by.append(src[m])
        dstl_by.append(dst[m] - c * cfg.NPC)
        dgi_by.append(deginv[dst[m]])

    sched1, pc1 = build_layer_schedule(cfg, src_by, dstl_by, dgi_by, cfg.XROWS)

    # layer-2 local/remote split: edges whose src this core owns can gather
    # from p_self before the AllGather completes.  Per-tile local quotas come
    # from the cross-core min (rounded to the block grid) to minimize padding.
    TPC = cfg.TPC
    locm_by = [(src_by[c] // cfg.NPC) == c for c in range(NCORES)]
    tile_by = [dstl_by[c] // P for c in range(NCORES)]
    loc_cnt = np.zeros((NCORES, TPC), np.int64)
    for c in range(NCORES):
        np.add.at(loc_cnt[c], tile_by[c][locm_by[c]], 1)
    qloc = np.maximum(P, (np.rint(loc_cnt.min(axis=0) / P) * P)).astype(np.int64)
    srcL_by, dstlL_by, dgiL_by = [], [], []
    src2_by, dstlR_by, dgiR_by = [], [], []
    for c in range(NCORES):
        sel = np.zeros(len(src_by[c]), bool)
        for t in range(TPC):
            e = np.nonzero(locm_by[c] & (tile_by[c] == t))[0]
            take = min(len(e), int(qloc[t]))
            sel[e[:take]] = True
        srcL_by.append(src_by[c][sel] - c * cfg.NPC)
        dstlL_by.append(dstl_by[c][sel])
        dgiL_by.append(dgi_by[c][sel])
        r = ~sel
        so = src_by[c][r] // cfg.NPC
        src2_by.append(so * cfg.NPCP + (src_by[c][r] - so * cfg.NPC))
        dstlR_by.append(dstl_by[c][r])
        dgiR_by.append(dgi_by[c][r])

    schedL, pcL = build_layer_schedule(cfg, srcL_by, dstlL_by, dgiL_by,
                                       cfg.NPCP, nseg=1)
    sched2, pc2 = build_layer_schedule(cfg, src2_by, dstlR_by, dgiR_by,
                                       cfg.PROWS)

    iota = np.broadcast_to(np.arange(P), (P, P)).astype(BF)
    shared = dict(
        x_tbl=np.ascontiguousarray(np.asarray(x, np.float32).astype(BF)),
        w1lt=np.ascontiguousarray(W1_l.T, np.float32),
        w1rt=np.ascontiguousarray(W1_r.T, np.float32),
        b1=np.ascontiguousarray(b1_.reshape(-1, 1), np.float32),
        w2lt=np.ascontiguousarray(W2_l.T, np.float32),
        w2rt=np.ascontiguousarray(W2_r.T, np.float32),
        b2=np.ascontiguousarray(b2_.reshape(-1, 1), np.float32),
        wdt=np.ascontiguousarray(W_dec.T, np.float32),
        bd=np.ascontiguousarray(b_dec.reshape(-1, 1), np.float32),
        iota_in=iota,
    )
    in_maps = []
    for c in range(NCORES):
        xs = np.zeros((cfg.IN, cfg.NPCP), np.float32)
        xs[:, :cfg.NPC] = x[c * cfg.NPC:(c + 1) * cfg.NPC].T
        m = dict(shared)
        m.update(
            xT_c=xs,
            idx1=pc1[c]["idx16"], doff1=pc1[c]["dstoff"].astype(BF),
            scal1=pc1[c]["scale"].astype(BF),
            idx2=pc2[c]["idx16"], doff2=pc2[c]["dstoff"].astype(BF),
            scal2=pc2[c]["scale"].astype(BF),
            idxL=pcL[c]["idx16"], doffL=pcL[c]["dstoff"].astype(BF),
            scalL=pcL[c]["scale"].astype(BF),
        )
        in_maps.append(m)
    return sched1, sched2, schedL, in_maps


def assemble(results, cfg):
    xr = np.empty((cfg.N, cfg.IN), np.float32)
    z = np.empty((cfg.N, cfg.LAT), np.float32)
    for c in range(NCORES):
        xr[c * cfg.NPC:(c + 1) * cfg.NPC] = results[c]["xrT_out"][:, :cfg.NPC].T
        z[c * cfg.NPC:(c + 1) * cfg.NPC] = results[c]["zT_out"][:, :cfg.NPC].T
    return xr, z


def kernel(x, edge_index, W1_l, b1_l, W1_r, W2_l, b2_l, W2_r, W_dec, b_dec,
           _trace=False):
    x = np.asarray(x)
    edge_index = np.asarray(edge_index)
    n, e = x.shape[0], edge_index.shape[1]
    cfg = Cfg(n, e, x.shape[1], W1_l.shape[0], W2_l.shape[0],
              n // NCORES, gsize=7 if n == 100000 else 2)
    sched1, sched2, schedL, in_maps = prepare(
        x, edge_index, np.asarray(W1_l), np.asarray(b1_l), np.asarray(W1_r),
        np.asarray(W2_l), np.asarray(b2_l), np.asarray(W2_r),
        np.asarray(W_dec), np.asarray(b_dec), cfg)

    nc = build_bass(cfg, sched1, sched2, schedL)
    res = run_bass_kernel_spmd(nc, in_maps, core_ids=list(range(NCORES)),
                               trace=_trace)
    xr, z = assemble(res.results, cfg)
    kernel.last_exec_ns = res.exec_time_ns
    return (xr, z)


# revision 26
# speedup vs baseline: 1.2123x; 1.0956x over previous
"""Distributed Trainium2 (8 NeuronCores) kernel for a 2-layer GraphSAGE
autoencoder (mean aggregation) — nn_AnomalyDetector.

Strategy (vertex-cut by destination owner):
  - Nodes are sharded contiguously across 8 cores (12500 each, padded to
    12544 = 98 tiles x 128 slots).  Each core owns the edges whose dst it
    owns; segment-sum is local.
  - Layer 1 gathers rows of the (replicated) x table with dma_gather
    (int16 indices -> 4 overlapping 32768-row table segments; per-edge
    segment assignment is waterfilled per destination tile, with per-run
    static block quotas shared by all 8 cores so the SPMD graph is
    identical).
  - Segment-sum is a PE matmul against a one-hot matrix built on DVE
    (is_equal vs an iota row), accumulated in PSUM per destination tile.
    Per-edge 1/deg scaling is folded into the gathered rows, so padding
    slots (scale 0) contribute nothing.
  - Feat-major layouts throughout: aggT [f,n] -> hT [h,n] -> p [n,l]
    (node-major via operand swap) + s2T [l,n].  p is AllGathered to the
    full [100352, 64] table for the layer-2 gather.  Outputs xrecT
    [128, 12544] and zT [64, 12544] are transposed/trimmed on the host.
"""

import os
import sys

for _p in ("/opt/trn_rl_repo", "/root/.axon_site/_ro/trn_rl_repo"):
    if os.path.isdir(_p) and _p not in sys.path:
        sys.path.append(_p)

import numpy as np
import ml_dtypes

BF = ml_dtypes.bfloat16

import concourse.bass as bass
import concourse.mybir as mybir
from concourse import bacc
from concourse.tile import TileContext
from concourse.bass_utils import run_bass_kernel_spmd
from concourse.library_config import mlp

F32 = mybir.dt.float32
BF16 = mybir.dt.bfloat16
I16 = mybir.dt.int16

NCORES = 8
P = 128          # partitions / block size / tile node count
SEG_SPAN = 32768  # int16 index reach
NSEG = 4
SCHUNK = 16      # blocks per one-hot build
CALL_BLKS = 64   # max blocks per dma_gather call


# ----------------------------------------------------------------- config

class Cfg:
    def __init__(self, n, e, in_ch, hid_ch, lat_ch, npc, gsize):
        assert n % NCORES == 0 or True
        self.N, self.E = n, e
        self.IN, self.HID, self.LAT = in_ch, hid_ch, lat_ch
        self.NPC = npc                        # real nodes per core
        self.NPCP = -(-npc // P) * P          # padded
        self.TPC = self.NPCP // P             # tiles per core
        self.GSIZE = gsize                    # tiles per group
        assert self.TPC % gsize == 0
        self.NG = self.TPC // gsize
        # table sizes
        self.XROWS = n                        # layer-1 table rows
        self.PROWS = self.NPCP * NCORES       # layer-2 table rows

    def seg_bases(self, rows, nseg=NSEG):
        if nseg == 1 or rows <= SEG_SPAN:
            return [0] * nseg
        step = -(-(rows - SEG_SPAN) // (nseg - 1))
        return [min(s * step, rows - SEG_SPAN) for s in range(nseg)]


# ---------------------------------------------------------- host schedule

def _intervals(bases, rows):
    """Split [0, rows) into intervals with constant admissible-segment sets.
    Returns (bounds, adm) where bounds has len nivl+1 and adm[i] is the
    ordered list of admissible segs for interval i."""
    nseg = len(bases)
    lo = np.array(bases)
    hi = np.minimum(lo + SEG_SPAN, rows)
    cuts = np.unique(np.concatenate([lo, hi, [0, rows]]))
    cuts = cuts[(cuts >= 0) & (cuts <= rows)]
    adm = []
    bounds = []
    for i in range(len(cuts) - 1):
        a, b = cuts[i], cuts[i + 1]
        if a == b:
            continue
        mid = (a + b) // 2
        segs = [s for s in range(nseg) if lo[s] <= mid < hi[s]]
        assert segs, f"uncovered interval [{a},{b})"
        bounds.append(a)
        adm.append(segs)
    bounds.append(rows)
    return np.array(bounds), adm


def _waterfill_counts(counts, adm, nseg=NSEG):
    """counts[i] edges in interval i with admissible segs adm[i].
    Returns take[i][s] = how many of interval i go to seg s.

    Fills each non-final segment up to a multiple of P (block grid) so the
    per-(tile,seg) quotas waste as little block padding as possible; the
    remainder lands in the last admissible segment."""
    total = int(counts.sum())
    # baseline grid target for segments with no mandatory load yet
    gtarget = max(P, (total // nseg) // P * P)
    loads = np.zeros(nseg, np.int64)
    take = []
    for i, n in enumerate(counts):
        n = int(n)
        segs = adm[i]
        tk = {}
        for s in segs[:-1]:
            # top this segment up to its own next block-grid point (or the
            # baseline target) so its quota has no padding; spill the rest
            tgt = max(-(-int(loads[s]) // P) * P, gtarget)
            a = int(np.clip(tgt - loads[s], 0, n))
            tk[s] = a
            loads[s] += a
            n -= a
        tk[segs[-1]] = n
        loads[segs[-1]] += n
        take.append(tk)
    return take


def build_layer_schedule(cfg, src_by_core, dst_local_by_core, deginv_by_core,
                         table_rows, nseg=NSEG):
    """Compute the SPMD-static schedule + per-core slot arrays for one layer.

    src_by_core[c]: global table row per edge (int64)
    dst_local_by_core[c]: local node id (0..NPC-1) per edge
    deginv_by_core[c]: f32 scale per edge
    Returns (sched, percore) where sched is shared and percore is a list of
    dicts with device-ready arrays.
    """
    bases = cfg.seg_bases(table_rows, nseg)
    TPC = cfg.TPC
    bounds, adm = _intervals(bases, table_rows)
    nivl = len(adm)

    # per (core, tile, seg) edge index lists
    per_cts = [[[None] * nseg for _ in range(TPC)] for _ in range(NCORES)]
    for c in range(NCORES):
        src = src_by_core[c]
        dstl = dst_local_by_core[c]
        tile = (dstl // P).astype(np.int64)
        iv = np.searchsorted(bounds, src, side="right") - 1
        order = np.lexsort((iv, tile))
        # counts matrix [TPC, nivl]
        cnt = np.zeros((TPC, nivl), np.int64)
        np.add.at(cnt, (tile, iv), 1)
        # prefix offsets of each (t, iv) run in `order`
        run_off = np.zeros((TPC, nivl), np.int64)
        flat = cnt.reshape(-1).cumsum()
        run_off.reshape(-1)[1:] = flat[:-1]
        for t in range(TPC):
            take = _waterfill_counts(cnt[t], adm, nseg)
            buckets = {s: [] for s in range(nseg)}
            for i in range(nivl):
                o = int(run_off[t, i])
                n = int(cnt[t, i])
                pos = 0
                for s, k in take[i].items():
                    if k:
                        buckets[s].append(order[o + pos:o + pos + k])
                        pos += k
            for s in range(nseg):
                per_cts[c][t][s] = (np.concatenate(buckets[s])
                                    if buckets[s] else np.empty(0, np.int64))

    # static quotas (blocks) per (tile, seg): max over cores
    quota = np.zeros((TPC, nseg), np.int64)
    for t in range(TPC):
        for s in range(nseg):
            mx = max(len(per_cts[c][t][s]) for c in range(NCORES))
            quota[t, s] = -(-mx // P)

    # slot layout: [group][seg][tile in group][quota blocks]
    blocks = []           # list of (tile, seg) per block, in slot order
    calls = []            # list of (seg, blk_start, nblk)
    for g in range(cfg.NG):
        tiles = range(g * cfg.GSIZE, (g + 1) * cfg.GSIZE)
        for s in range(nseg):
            run_start = len(blocks)
            for t in tiles:
                for _ in range(quota[t, s]):
                    blocks.append((t, s))
            run_len = len(blocks) - run_start
            b0 = run_start
            while run_len > 0:
                nb = min(run_len, CALL_BLKS)
                calls.append((s, b0, nb))
                b0 += nb
                run_len -= nb
    totblk = len(blocks)
    sched = dict(bases=bases, quota=quota, blocks=blocks, calls=calls,
                 totblk=totblk)

    # per-core arrays
    percore = []
    for c in range(NCORES):
        idx_rel = np.zeros(totblk * P, np.int64)
        dstoff = np.zeros(totblk * P, np.float32)
        scale = np.zeros(totblk * P, np.float32)
        # fill slots tile/seg cell by cell
        cell_cursor = {}
        b_of_cell = {}
        pos = 0
        for bi, (t, s) in enumerate(blocks):
            b_of_cell.setdefault((t, s), []).append(bi)
        for (t, s), bl in b_of_cell.items():
            e = per_cts[c][t][s]
            slots = np.concatenate([np.arange(b * P, (b + 1) * P) for b in bl])
            k = len(e)
            assert k <= len(slots)
            src = src_by_core[c][e]
            idx_rel[slots[:k]] = src - bases[s]
            dstoff[slots[:k]] = (dst_local_by_core[c][e] % P).astype(np.float32)
            scale[slots[:k]] = deginv_by_core[c][e]
            # padding slots keep idx_rel 0 (valid row), scale 0
        assert idx_rel.min() >= 0 and idx_rel.max() < SEG_SPAN

        # [128, totblk] layouts (slot -> p = slot%128, k = slot//128)
        dstoff_t = dstoff.reshape(totblk, P).T.copy()
        scale_t = scale.reshape(totblk, P).T.copy()

        # int16 idx array packed per call: for call (s, b0, nb):
        # within-call j -> [j%16, col j//16], replicated over 8 groups,
        # stored at columns [b0*8, (b0+nb)*8)
        idx16 = np.zeros((P, totblk * 8), np.int16)
        for (s, b0, nb) in calls:
            n = nb * P
            j = np.arange(n)
            vals = idx_rel[b0 * P: b0 * P + n].astype(np.int16)
            cols = b0 * 8 + j // 16
            for grp in range(8):
                idx16[grp * 16 + (j % 16), cols] = vals
        percore.append(dict(idx16=idx16, dstoff=dstoff_t, scale=scale_t))
    return sched, percore


# ------------------------------------------------------------- bass build

def emit_gather_segsum(nc, tc, pools, cfg, sched, elem, tbl, idx_res, doff_res,
                       scal_res, psum_pool, psum_shape, evict_fn, qstate,
                       iota_t, gdt=F32, post_call_hooks=None):
    """Emit one layer's gather + scaled one-hot segment-sum.

    evict_fn(t, psum_tile) is called once per tile after its accumulation
    completes.
    """
    gpool, spool = pools
    blocks, calls, quota = sched["blocks"], sched["calls"], sched["quota"]
    bases = sched["bases"]
    # first/last block index per tile
    first_blk, last_blk = {}, {}
    for bi, (t, s) in enumerate(blocks):
        first_blk.setdefault(t, bi)
        last_blk[t] = bi
    psum_tiles = {}

    for ci, (s, b0, nb) in enumerate(calls):
        n_idx = nb * P
        G = gpool.tile([P, CALL_BLKS * elem], gdt, tag="G")
        lo = bases[s]
        hi = min(lo + SEG_SPAN, tbl.shape[0])
        nc.gpsimd.dma_gather(
            G[:].rearrange("p (k d) -> p k d", d=elem)[:, :nb, :],
            tbl[lo:hi, :],
            idx_res[:, b0 * 8:(b0 + nb) * 8],
            n_idx, n_idx, elem,
            single_packet=False,
            queue_num=qstate[0],
        )
        qstate[0] = (qstate[0] + 1) % 4
        # per-edge scale (also zeroes padding slots); bf16 for the PE
        GS = gpool.tile([P, CALL_BLKS * elem], BF16, tag="GS")
        G3 = G[:].rearrange("p (k d) -> p k d", d=elem)[:, :nb, :]
        GS3 = GS[:].rearrange("p (k d) -> p k d", d=elem)[:, :nb, :]
        nc.vector.tensor_tensor(
            out=GS3, in0=G3,
            in1=doff_scale_bcast(scal_res, b0, nb, elem),
            op=mybir.AluOpType.mult,
        )
        # one-hot chunks + matmuls
        for sc0 in range(0, nb, SCHUNK):
            scn = min(SCHUNK, nb - sc0)
            S = spool.tile([P, SCHUNK * P], BF16, tag="S")
            S3 = S[:].rearrange("p (k n) -> p k n", n=P)[:, :scn, :]
            nc.vector.tensor_tensor(
                out=S3,
                in0=doff_scale_bcast(doff_res, b0 + sc0, scn, P),
                in1=iota_t[:].rearrange("p (a n) -> p a n", a=1)
                    .to_broadcast([P, scn, P]),
                op=mybir.AluOpType.is_equal,
            )
            for k in range(scn):
                bi = b0 + sc0 + k
                t, _s = blocks[bi]
                if t not in psum_tiles:
                    seg_psum = psum_pool.tile(list(psum_shape), F32, tag="seg")
                    psum_tiles[t] = seg_psum
                nc.tensor.matmul(
                    psum_tiles[t][:],
                    lhsT=GS[:, (sc0 + k) * elem:(sc0 + k + 1) * elem],
                    rhs=S[:, k * P:(k + 1) * P],
                    start=(bi == first_blk[t]),
                    stop=(bi == last_blk[t]),
                )
                if bi == last_blk[t]:
                    evict_fn(t, psum_tiles.pop(t))
        if post_call_hooks and ci in post_call_hooks:
            post_call_hooks[ci]()


def doff_scale_bcast(res, k0, nb, width):
    return res[:, k0:k0 + nb].to_broadcast([P, nb, width])


def build_bass(cfg, sched1, sched2, schedL, compile_=True):
    nc = bacc.Bacc("TRN2", target_bir_lowering=False, debug=False,
                   num_devices=NCORES, num_swdge_queues=4)
    IN, HID, LAT = cfg.IN, cfg.HID, cfg.LAT
    NPCP, TPC = cfg.NPCP, cfg.TPC

    x_tbl = nc.declare_dram_parameter("x_tbl", [cfg.XROWS, IN], BF16, isOutput=False)
    xT_c = nc.declare_dram_parameter("xT_c", [IN, NPCP], F32, isOutput=False)
    idx1 = nc.declare_dram_parameter("idx1", [P, sched1["totblk"] * 8], I16, isOutput=False)
    doff1 = nc.declare_dram_parameter("doff1", [P, sched1["totblk"]], BF16, isOutput=False)
    scal1 = nc.declare_dram_parameter("scal1", [P, sched1["totblk"]], BF16, isOutput=False)
    idx2 = nc.declare_dram_parameter("idx2", [P, sched2["totblk"] * 8], I16, isOutput=False)
    if schedL is not None:
        idxL = nc.declare_dram_parameter("idxL", [P, schedL["totblk"] * 8], I16, isOutput=False)
        doffL = nc.declare_dram_parameter("doffL", [P, schedL["totblk"]], BF16, isOutput=False)
        scalL = nc.declare_dram_parameter("scalL", [P, schedL["totblk"]], BF16, isOutput=False)
    doff2 = nc.declare_dram_parameter("doff2", [P, sched2["totblk"]], BF16, isOutput=False)
    scal2 = nc.declare_dram_parameter("scal2", [P, sched2["totblk"]], BF16, isOutput=False)
    w1lt = nc.declare_dram_parameter("w1lt", [IN, HID], F32, isOutput=False)
    w1rt = nc.declare_dram_parameter("w1rt", [IN, HID], F32, isOutput=False)
    b1 = nc.declare_dram_parameter("b1", [HID, 1], F32, isOutput=False)
    w2lt = nc.declare_dram_parameter("w2lt", [HID, LAT], F32, isOutput=False)
    w2rt = nc.declare_dram_parameter("w2rt", [HID, LAT], F32, isOutput=False)
    b2 = nc.declare_dram_parameter("b2", [LAT, 1], F32, isOutput=False)
    wdt = nc.declare_dram_parameter("wdt", [LAT, IN], F32, isOutput=False)
    bd = nc.declare_dram_parameter("bd", [IN, 1], F32, isOutput=False)
    iota_in = nc.declare_dram_parameter("iota_in", [P, P], BF16, isOutput=False)

    zT_out = nc.declare_dram_parameter("zT_out", [LAT, NPCP], F32, isOutput=True)
    xrT_out = nc.declare_dram_parameter("xrT_out", [IN, NPCP], F32, isOutput=True)

    p_self = nc.dram_tensor("p_self", [NPCP, LAT], F32)
    msg2loc = nc.dram_tensor("msg2loc", [LAT, NPCP], F32)
    p_full = nc.dram_tensor("p_full", [cfg.PROWS, LAT], F32, addr_space="Shared")
    s2_dram = nc.dram_tensor("s2_dram", [LAT, NPCP], F32)

    with TileContext(nc) as tc:
        with tc.tile_pool(name="const", bufs=1) as cpool, \
             tc.tile_pool(name="gpool", bufs=4) as gpool, \
             tc.tile_pool(name="spool", bufs=4) as spool, \
             tc.tile_pool(name="meta", bufs=1) as mpool, \
             tc.tile_pool(name="stage", bufs=3) as stpool, \
             tc.tile_pool(name="psum", bufs=8, space="PSUM") as psum_pool:

            nc.gpsimd.load_library(mlp)

            # constants
            iota_t = cpool.tile([P, P], BF16)
            nc.sync.dma_start(out=iota_t[:], in_=iota_in[:, :])
            w1lt_t = cpool.tile([IN, HID], F32)
            nc.sync.dma_start(out=w1lt_t[:], in_=w1lt[:, :])
            w1rt_t = cpool.tile([IN, HID], F32)
            nc.sync.dma_start(out=w1rt_t[:], in_=w1rt[:, :])
            b1_t = cpool.tile([HID, 1], F32)
            nc.sync.dma_start(out=b1_t[:], in_=b1[:, :])
            w2lt_t = cpool.tile([HID, LAT], F32)
            nc.sync.dma_start(out=w2lt_t[:], in_=w2lt[:, :])
            w2rt_t = cpool.tile([HID, LAT], F32)
            nc.sync.dma_start(out=w2rt_t[:], in_=w2rt[:, :])
            b2_t = cpool.tile([LAT, 1], F32)
            nc.sync.dma_start(out=b2_t[:], in_=b2[:, :])
            wdt_t = cpool.tile([LAT, IN], F32)
            nc.sync.dma_start(out=wdt_t[:], in_=wdt[:, :])
            bd_t = cpool.tile([IN, 1], F32)
            nc.sync.dma_start(out=bd_t[:], in_=bd[:, :])

            # resident metadata, layer 1
            idx1_t = mpool.tile([P, sched1["totblk"] * 8], I16, tag="idx")
            nc.sync.dma_start(out=idx1_t[:], in_=idx1[:, :])
            doff1_t = mpool.tile([P, sched1["totblk"]], BF16)
            nc.sync.dma_start(out=doff1_t[:], in_=doff1[:, :])
            scal1_t = mpool.tile([P, sched1["totblk"]], BF16)
            nc.sync.dma_start(out=scal1_t[:], in_=scal1[:, :])

            qstate = [0]

            def evict1(t, psum_t):
                c0, c1 = t * P, (t + 1) * P
                aggT = stpool.tile([IN, P], F32, tag="aggT")
                nc.vector.tensor_copy(out=aggT[:], in_=psum_t[:])
                xT_t = stpool.tile([IN, P], F32, tag="xTt")
                nc.sync.dma_start(out=xT_t[:], in_=xT_c[:, c0:c1])
                hps = psum_pool.tile([HID, P], F32, tag="seg")
                nc.tensor.matmul(hps[:], lhsT=w1lt_t[:], rhs=aggT[:],
                                 start=True, stop=False)
                nc.tensor.matmul(hps[:], lhsT=w1rt_t[:], rhs=xT_t[:],
                                 start=False, stop=True)
                hT = stpool.tile([HID, P], F32, tag="hT")
                nc.scalar.activation(hT[:], hps[:],
                                     mybir.ActivationFunctionType.Relu,
                                     bias=b1_t[:])
                pns = psum_pool.tile([P, LAT], F32, tag="seg")
                nc.tensor.matmul(pns[:], lhsT=hT[:], rhs=w2lt_t[:],
                                 start=True, stop=True)
                s2s = psum_pool.tile([LAT, P], F32, tag="seg")
                nc.tensor.matmul(s2s[:], lhsT=w2rt_t[:], rhs=hT[:],
                                 start=True, stop=True)
                pn = stpool.tile([P, LAT], F32, tag="pn")
                nc.vector.tensor_copy(out=pn[:], in_=pns[:])
                s2 = stpool.tile([LAT, P], F32, tag="s2")
                nc.vector.tensor_scalar_add(s2[:], s2s[:], b2_t[:])
                nc.sync.dma_start(out=p_self[c0:c1, :], in_=pn[:])
                nc.sync.dma_start(out=s2_dram[:, c0:c1], in_=s2[:])

            emit_gather_segsum(nc, tc, (gpool, spool), cfg, sched1, IN,
                               x_tbl, idx1_t[:], doff1_t[:], scal1_t[:],
                               psum_pool, (IN, P), evict1, qstate, iota_t,
                               gdt=BF16)

            # AllGather p (trigger waits on p_self, then CC runs in
            # parallel with the local-edge gather phase below)
            nc.gpsimd.collective_compute(
                "AllGather", mybir.AluOpType.bypass,
                replica_groups=[list(range(NCORES))],
                ins=[p_self.ap().opt()],
                outs=[p_full.ap().opt()],
            )

            # local-edge gather phase: own-core sources from p_self, no AG dep
            if schedL is not None:
                idxL_t = mpool.tile([P, schedL["totblk"] * 8], I16)
                nc.sync.dma_start(out=idxL_t[:], in_=idxL[:, :])
                doffL_t = mpool.tile([P, schedL["totblk"]], BF16)
                nc.sync.dma_start(out=doffL_t[:], in_=doffL[:, :])
                scalL_t = mpool.tile([P, schedL["totblk"]], BF16)
                nc.sync.dma_start(out=scalL_t[:], in_=scalL[:, :])

                def evict_loc(t, psum_t):
                    c0, c1 = t * P, (t + 1) * P
                    locT = stpool.tile([LAT, P], F32, tag="locT")
                    nc.vector.tensor_copy(out=locT[:], in_=psum_t[:])
                    nc.sync.dma_start(out=msg2loc[:, c0:c1], in_=locT[:])

                emit_gather_segsum(nc, tc, (gpool, spool), cfg, schedL, LAT,
                                   p_self, idxL_t[:], doffL_t[:], scalL_t[:],
                                   psum_pool, (LAT, P), evict_loc, qstate,
                                   iota_t)

            # resident metadata, layer 2
            idx2_t = mpool.tile([P, sched2["totblk"] * 8], I16, tag="idx")
            nc.sync.dma_start(out=idx2_t[:], in_=idx2[:, :])
            doff2_t = mpool.tile([P, sched2["totblk"]], BF16)
            nc.sync.dma_start(out=doff2_t[:], in_=doff2[:, :])
            scal2_t = mpool.tile([P, sched2["totblk"]], BF16)
            nc.sync.dma_start(out=scal2_t[:], in_=scal2[:, :])

            def evict2(t, psum_t):
                c0, c1 = t * P, (t + 1) * P
                s2t = stpool.tile([LAT, P], F32, tag="s2t")
                nc.sync.dma_start(out=s2t[:], in_=s2_dram[:, c0:c1])
                zT = stpool.tile([LAT, P], F32, tag="zT")
                nc.vector.tensor_add(out=zT[:], in0=psum_t[:], in1=s2t[:])
                if schedL is not None:
                    loct = stpool.tile([LAT, P], F32, tag="loct")
                    nc.sync.dma_start(out=loct[:], in_=msg2loc[:, c0:c1])
                    nc.vector.tensor_add(out=zT[:], in0=zT[:], in1=loct[:])
                xrp = psum_pool.tile([IN, P], F32, tag="seg")
                nc.tensor.matmul(xrp[:], lhsT=wdt_t[:], rhs=zT[:],
                                 start=True, stop=True)
                xr = stpool.tile([IN, P], F32, tag="xr")
                nc.vector.tensor_scalar_add(xr[:], xrp[:], bd_t[:])
                nc.sync.dma_start(out=zT_out[:, c0:c1], in_=zT[:])
                nc.sync.dma_start(out=xrT_out[:, c0:c1], in_=xr[:])

            emit_gather_segsum(nc, tc, (gpool, spool), cfg, sched2, LAT,
                               p_full, idx2_t[:], doff2_t[:], scal2_t[:],
                               psum_pool, (LAT, P), evict2, qstate, iota_t)

    if compile_:
        nc.compile()
    return nc


# ------------------------------------------------------------ host driver

def prepare(x, edge_index, W1_l, b1_, W1_r, W2_l, b2_, W2_r, W_dec, b_dec,
            cfg):
    src = np.asarray(edge_index[0], np.int64)
    dst = np.asarray(edge_index[1], np.int64)
    N, E = cfg.N, cfg.E
    deg = np.bincount(dst, minlength=N).astype(np.float32)
    deginv = 1.0 / np.maximum(deg, 1.0)

    owner = dst // cfg.NPC
    src_by, dstl_by, dgi_by = [], [], []
    for c in range(NCORES):
        m = np.nonzero(owner == c)[0]
        src_by.append(src[m])
        dstl_by.append(dst[m] - c * cfg.NPC)
        dgi_by.append(deginv[dst[m]])

    sched1, pc1 = build_layer_schedule(cfg, src_by, dstl_by, dgi_by, cfg.XROWS)

    # layer-2 local/remote split: edges whose src this core owns can gather
    # from p_self before the AllGather completes.  Per-tile local quotas come
    # from the cross-core min (rounded to the block grid) to minimize padding.
    TPC = cfg.TPC
    locm_by = [(src_by[c] // cfg.NPC) == c for c in range(NCORES)]
    tile_by = [dstl_by[c] // P for c in range(NCORES)]
    loc_cnt = np.zeros((NCORES, TPC), np.int64)
    for c in range(NCORES):
        np.add.at(loc_cnt[c], tile_by[c][locm_by[c]], 1)
    qloc = np.zeros(TPC, np.int64)  # local phase disabled (hole inflation)
    srcL_by, dstlL_by, dgiL_by = [], [], []
    src2_by, dstlR_by, dgiR_by = [], [], []
    for c in range(NCORES):
        sel = np.zeros(len(src_by[c]), bool)
        for t in range(TPC):
            e = np.nonzero(locm_by[c] & (tile_by[c] == t))[0]
            take = min(len(e), int(qloc[t]))
            sel[e[:take]] = True
        srcL_by.append(src_by[c][sel] - c * cfg.NPC)
        dstlL_by.append(dstl_by[c][sel])
        dgiL_by.append(dgi_by[c][sel])
        r = ~sel
        so = src_by[c][r] // cfg.NPC
        src2_by.append(so * cfg.NPCP + (src_by[c][r] - so * cfg.NPC))
        dstlR_by.append(dstl_by[c][r])
        dgiR_by.append(dgi_by[c][r])

    use_local = int(qloc.sum()) > 0
    if use_local:
        schedL, pcL = build_layer_schedule(cfg, srcL_by, dstlL_by, dgiL_by,
                                           cfg.NPCP, nseg=1)
    else:
        schedL, pcL = None, None
    sched2, pc2 = build_layer_schedule(cfg, src2_by, dstlR_by, dgiR_by,
                                       cfg.PROWS)

    iota = np.broadcast_to(np.arange(P), (P, P)).astype(BF)
    shared = dict(
        x_tbl=np.ascontiguousarray(np.asarray(x, np.float32).astype(BF)),
        w1lt=np.ascontiguousarray(W1_l.T, np.float32),
        w1rt=np.ascontiguousarray(W1_r.T, np.float32),
        b1=np.ascontiguousarray(b1_.reshape(-1, 1), np.float32),
        w2lt=np.ascontiguousarray(W2_l.T, np.float32),
        w2rt=np.ascontiguousarray(W2_r.T, np.float32),
        b2=np.ascontiguousarray(b2_.reshape(-1, 1), np.float32),
        wdt=np.ascontiguousarray(W_dec.T, np.float32),
        bd=np.ascontiguousarray(b_dec.reshape(-1, 1), np.float32),
        iota_in=iota,
    )
    in_maps = []
    for c in range(NCORES):
        xs = np.zeros((cfg.IN, cfg.NPCP), np.float32)
        xs[:, :cfg.NPC] = x[c * cfg.NPC:(c + 1) * cfg.NPC].T
        m = dict(shared)
        m.update(
            xT_c=xs,
            idx1=pc1[c]["idx16"], doff1=pc1[c]["dstoff"].astype(BF),
            scal1=pc1[c]["scale"].astype(BF),
            idx2=pc2[c]["idx16"], doff2=pc2[c]["dstoff"].astype(BF),
            scal2=pc2[c]["scale"].astype(BF),
            **({"idxL": pcL[c]["idx16"], "doffL": pcL[c]["dstoff"].astype(BF),
                "scalL": pcL[c]["scale"].astype(BF)} if pcL else {}),
        )
        in_maps.append(m)
    return sched1, sched2, schedL, in_maps


def assemble(results, cfg):
    xr = np.empty((cfg.N, cfg.IN), np.float32)
    z = np.empty((cfg.N, cfg.LAT), np.float32)
    for c in range(NCORES):
        xr[c * cfg.NPC:(c + 1) * cfg.NPC] = results[c]["xrT_out"][:, :cfg.NPC].T
        z[c * cfg.NPC:(c + 1) * cfg.NPC] = results[c]["zT_out"][:, :cfg.NPC].T
    return xr, z


def kernel(x, edge_index, W1_l, b1_l, W1_r, W2_l, b2_l, W2_r, W_dec, b_dec,
           _trace=False):
    x = np.asarray(x)
    edge_index = np.asarray(edge_index)
    n, e = x.shape[0], edge_index.shape[1]
    cfg = Cfg(n, e, x.shape[1], W1_l.shape[0], W2_l.shape[0],
              n // NCORES, gsize=7 if n == 100000 else 2)
    sched1, sched2, schedL, in_maps = prepare(
        x, edge_index, np.asarray(W1_l), np.asarray(b1_l), np.asarray(W1_r),
        np.asarray(W2_l), np.asarray(b2_l), np.asarray(W2_r),
        np.asarray(W_dec), np.asarray(b_dec), cfg)

    nc = build_bass(cfg, sched1, sched2, schedL)
    res = run_bass_kernel_spmd(nc, in_maps, core_ids=list(range(NCORES)),
                               trace=_trace)
    xr, z = assemble(res.results, cfg)
    kernel.last_exec_ns = res.exec_time_ns
    return (xr, z)


# revision 27
# speedup vs baseline: 1.4425x; 1.1900x over previous
"""Distributed Trainium2 (8 NeuronCores) kernel for a 2-layer GraphSAGE
autoencoder (mean aggregation) — nn_AnomalyDetector.

Strategy (vertex-cut by destination owner):
  - Nodes are sharded contiguously across 8 cores (12500 each, padded to
    12544 = 98 tiles x 128 slots).  Each core owns the edges whose dst it
    owns; segment-sum is local.
  - Layer 1 gathers rows of the (replicated) x table with dma_gather
    (int16 indices -> 4 overlapping 32768-row table segments; per-edge
    segment assignment is waterfilled per destination tile, with per-run
    static block quotas shared by all 8 cores so the SPMD graph is
    identical).
  - Segment-sum is a PE matmul against a one-hot matrix built on DVE
    (is_equal vs an iota row), accumulated in PSUM per destination tile.
    Per-edge 1/deg scaling is folded into the gathered rows, so padding
    slots (scale 0) contribute nothing.
  - Feat-major layouts throughout: aggT [f,n] -> hT [h,n] -> p [n,l]
    (node-major via operand swap) + s2T [l,n].  p is AllGathered to the
    full [100352, 64] table for the layer-2 gather.  Outputs xrecT
    [128, 12544] and zT [64, 12544] are transposed/trimmed on the host.
"""

import os
import sys

for _p in ("/opt/trn_rl_repo", "/root/.axon_site/_ro/trn_rl_repo"):
    if os.path.isdir(_p) and _p not in sys.path:
        sys.path.append(_p)

import numpy as np
import ml_dtypes

BF = ml_dtypes.bfloat16

import concourse.bass as bass
import concourse.mybir as mybir
from concourse import bacc
from concourse.tile import TileContext
from concourse.bass_utils import run_bass_kernel_spmd
from concourse.library_config import mlp

F32 = mybir.dt.float32
BF16 = mybir.dt.bfloat16
I16 = mybir.dt.int16

NCORES = 8
P = 128          # partitions / block size / tile node count
SEG_SPAN = 32768  # int16 index reach
NSEG = 4
SCHUNK = 16      # blocks per one-hot build
CALL_BLKS = 64   # max blocks per dma_gather call


# ----------------------------------------------------------------- config

class Cfg:
    def __init__(self, n, e, in_ch, hid_ch, lat_ch, npc, gsize):
        assert n % NCORES == 0 or True
        self.N, self.E = n, e
        self.IN, self.HID, self.LAT = in_ch, hid_ch, lat_ch
        self.NPC = npc                        # real nodes per core
        self.NPCP = -(-npc // P) * P          # padded
        self.TPC = self.NPCP // P             # tiles per core
        self.GSIZE = gsize                    # tiles per group
        # ragged group list: full-size groups, then a small tail pair so the
        # final eviction cascade (which gates the AllGather) is short
        groups = []
        t = 0
        while self.TPC - t > gsize + 3:
            groups.append(range(t, t + gsize))
            t += gsize
        rem = self.TPC - t
        if rem > 3:
            groups.append(range(t, t + rem - 3))
            groups.append(range(self.TPC - 3, self.TPC))
        elif rem > 0:
            groups.append(range(t, self.TPC))
        self.groups = groups
        self.NG = len(groups)
        # table sizes
        self.XROWS = n                        # layer-1 table rows
        self.PROWS = self.NPCP * NCORES       # layer-2 table rows

    def seg_bases(self, rows, nseg=NSEG):
        if nseg == 1 or rows <= SEG_SPAN:
            return [0] * nseg
        step = -(-(rows - SEG_SPAN) // (nseg - 1))
        return [min(s * step, rows - SEG_SPAN) for s in range(nseg)]


# ---------------------------------------------------------- host schedule

def _intervals(bases, rows):
    """Split [0, rows) into intervals with constant admissible-segment sets.
    Returns (bounds, adm) where bounds has len nivl+1 and adm[i] is the
    ordered list of admissible segs for interval i."""
    nseg = len(bases)
    lo = np.array(bases)
    hi = np.minimum(lo + SEG_SPAN, rows)
    cuts = np.unique(np.concatenate([lo, hi, [0, rows]]))
    cuts = cuts[(cuts >= 0) & (cuts <= rows)]
    adm = []
    bounds = []
    for i in range(len(cuts) - 1):
        a, b = cuts[i], cuts[i + 1]
        if a == b:
            continue
        mid = (a + b) // 2
        segs = [s for s in range(nseg) if lo[s] <= mid < hi[s]]
        assert segs, f"uncovered interval [{a},{b})"
        bounds.append(a)
        adm.append(segs)
    bounds.append(rows)
    return np.array(bounds), adm


def _waterfill_counts(counts, adm, nseg=NSEG):
    """counts[i] edges in interval i with admissible segs adm[i].
    Returns take[i][s] = how many of interval i go to seg s.

    Fills each non-final segment up to a multiple of P (block grid) so the
    per-(tile,seg) quotas waste as little block padding as possible; the
    remainder lands in the last admissible segment."""
    total = int(counts.sum())
    # baseline grid target for segments with no mandatory load yet
    gtarget = max(P, (total // nseg) // P * P)
    loads = np.zeros(nseg, np.int64)
    take = []
    for i, n in enumerate(counts):
        n = int(n)
        segs = adm[i]
        tk = {}
        for s in segs[:-1]:
            # top this segment up to its own next block-grid point (or the
            # baseline target) so its quota has no padding; spill the rest
            tgt = max(-(-int(loads[s]) // P) * P, gtarget)
            a = int(np.clip(tgt - loads[s], 0, n))
            tk[s] = a
            loads[s] += a
            n -= a
        tk[segs[-1]] = n
        loads[segs[-1]] += n
        take.append(tk)
    return take


def build_layer_schedule(cfg, src_by_core, dst_local_by_core, deginv_by_core,
                         table_rows, nseg=NSEG):
    """Compute the SPMD-static schedule + per-core slot arrays for one layer.

    src_by_core[c]: global table row per edge (int64)
    dst_local_by_core[c]: local node id (0..NPC-1) per edge
    deginv_by_core[c]: f32 scale per edge
    Returns (sched, percore) where sched is shared and percore is a list of
    dicts with device-ready arrays.
    """
    bases = cfg.seg_bases(table_rows, nseg)
    TPC = cfg.TPC
    bounds, adm = _intervals(bases, table_rows)
    nivl = len(adm)

    # per (core, tile, seg) edge index lists
    per_cts = [[[None] * nseg for _ in range(TPC)] for _ in range(NCORES)]
    for c in range(NCORES):
        src = src_by_core[c]
        dstl = dst_local_by_core[c]
        tile = (dstl // P).astype(np.int64)
        iv = np.searchsorted(bounds, src, side="right") - 1
        order = np.lexsort((iv, tile))
        # counts matrix [TPC, nivl]
        cnt = np.zeros((TPC, nivl), np.int64)
        np.add.at(cnt, (tile, iv), 1)
        # prefix offsets of each (t, iv) run in `order`
        run_off = np.zeros((TPC, nivl), np.int64)
        flat = cnt.reshape(-1).cumsum()
        run_off.reshape(-1)[1:] = flat[:-1]
        for t in range(TPC):
            take = _waterfill_counts(cnt[t], adm, nseg)
            buckets = {s: [] for s in range(nseg)}
            for i in range(nivl):
                o = int(run_off[t, i])
                n = int(cnt[t, i])
                pos = 0
                for s, k in take[i].items():
                    if k:
                        buckets[s].append(order[o + pos:o + pos + k])
                        pos += k
            for s in range(nseg):
                per_cts[c][t][s] = (np.concatenate(buckets[s])
                                    if buckets[s] else np.empty(0, np.int64))

    # static quotas (blocks) per (tile, seg): max over cores
    quota = np.zeros((TPC, nseg), np.int64)
    for t in range(TPC):
        for s in range(nseg):
            mx = max(len(per_cts[c][t][s]) for c in range(NCORES))
            quota[t, s] = -(-mx // P)

    # slot layout: [group][seg][tile in group][quota blocks]
    blocks = []           # list of (tile, seg) per block, in slot order
    calls = []            # list of (seg, blk_start, nblk)
    for tiles in cfg.groups:
        for s in range(nseg):
            run_start = len(blocks)
            for t in tiles:
                for _ in range(quota[t, s]):
                    blocks.append((t, s))
            run_len = len(blocks) - run_start
            b0 = run_start
            while run_len > 0:
                nb = min(run_len, CALL_BLKS)
                calls.append((s, b0, nb))
                b0 += nb
                run_len -= nb
    totblk = len(blocks)
    sched = dict(bases=bases, quota=quota, blocks=blocks, calls=calls,
                 totblk=totblk)

    # per-core arrays
    percore = []
    for c in range(NCORES):
        idx_rel = np.zeros(totblk * P, np.int64)
        dstoff = np.zeros(totblk * P, np.float32)
        scale = np.zeros(totblk * P, np.float32)
        # fill slots tile/seg cell by cell
        cell_cursor = {}
        b_of_cell = {}
        pos = 0
        for bi, (t, s) in enumerate(blocks):
            b_of_cell.setdefault((t, s), []).append(bi)
        for (t, s), bl in b_of_cell.items():
            e = per_cts[c][t][s]
            slots = np.concatenate([np.arange(b * P, (b + 1) * P) for b in bl])
            k = len(e)
            assert k <= len(slots)
            src = src_by_core[c][e]
            idx_rel[slots[:k]] = src - bases[s]
            dstoff[slots[:k]] = (dst_local_by_core[c][e] % P).astype(np.float32)
            scale[slots[:k]] = deginv_by_core[c][e]
            # padding slots keep idx_rel 0 (valid row), scale 0
        assert idx_rel.min() >= 0 and idx_rel.max() < SEG_SPAN

        # [128, totblk] layouts (slot -> p = slot%128, k = slot//128)
        dstoff_t = dstoff.reshape(totblk, P).T.copy()
        scale_t = scale.reshape(totblk, P).T.copy()

        # int16 idx array packed per call: for call (s, b0, nb):
        # within-call j -> [j%16, col j//16], replicated over 8 groups,
        # stored at columns [b0*8, (b0+nb)*8)
        idx16 = np.zeros((P, totblk * 8), np.int16)
        for (s, b0, nb) in calls:
            n = nb * P
            j = np.arange(n)
            vals = idx_rel[b0 * P: b0 * P + n].astype(np.int16)
            cols = b0 * 8 + j // 16
            for grp in range(8):
                idx16[grp * 16 + (j % 16), cols] = vals
        percore.append(dict(idx16=idx16, dstoff=dstoff_t, scale=scale_t))
    return sched, percore


# ------------------------------------------------------------- bass build

def emit_gather_segsum(nc, tc, pools, cfg, sched, elem, tbl, idx_res, doff_res,
                       scal_res, psum_pool, psum_shape, evict_fn, qstate,
                       iota_t, gdt=F32, post_call_hooks=None):
    """Emit one layer's gather + scaled one-hot segment-sum.

    evict_fn(t, psum_tile) is called once per tile after its accumulation
    completes.
    """
    gpool, spool = pools
    blocks, calls, quota = sched["blocks"], sched["calls"], sched["quota"]
    bases = sched["bases"]
    # first/last block index per tile
    first_blk, last_blk = {}, {}
    for bi, (t, s) in enumerate(blocks):
        first_blk.setdefault(t, bi)
        last_blk[t] = bi
    psum_tiles = {}

    for ci, (s, b0, nb) in enumerate(calls):
        n_idx = nb * P
        G = gpool.tile([P, CALL_BLKS * elem], gdt, tag="G")
        lo = bases[s]
        hi = min(lo + SEG_SPAN, tbl.shape[0])
        nc.gpsimd.dma_gather(
            G[:].rearrange("p (k d) -> p k d", d=elem)[:, :nb, :],
            tbl[lo:hi, :],
            idx_res[:, b0 * 8:(b0 + nb) * 8],
            n_idx, n_idx, elem,
            single_packet=False,
            queue_num=qstate[0],
        )
        qstate[0] = (qstate[0] + 1) % 4
        # per-edge scale (also zeroes padding slots); bf16 for the PE
        GS = gpool.tile([P, CALL_BLKS * elem], BF16, tag="GS")
        G3 = G[:].rearrange("p (k d) -> p k d", d=elem)[:, :nb, :]
        GS3 = GS[:].rearrange("p (k d) -> p k d", d=elem)[:, :nb, :]
        nc.vector.tensor_tensor(
            out=GS3, in0=G3,
            in1=doff_scale_bcast(scal_res, b0, nb, elem),
            op=mybir.AluOpType.mult,
        )
        # one-hot chunks + matmuls
        for sc0 in range(0, nb, SCHUNK):
            scn = min(SCHUNK, nb - sc0)
            S = spool.tile([P, SCHUNK * P], BF16, tag="S")
            S3 = S[:].rearrange("p (k n) -> p k n", n=P)[:, :scn, :]
            nc.vector.tensor_tensor(
                out=S3,
                in0=doff_scale_bcast(doff_res, b0 + sc0, scn, P),
                in1=iota_t[:].rearrange("p (a n) -> p a n", a=1)
                    .to_broadcast([P, scn, P]),
                op=mybir.AluOpType.is_equal,
            )
            for k in range(scn):
                bi = b0 + sc0 + k
                t, _s = blocks[bi]
                if t not in psum_tiles:
                    seg_psum = psum_pool.tile(list(psum_shape), F32, tag="seg")
                    psum_tiles[t] = seg_psum
                nc.tensor.matmul(
                    psum_tiles[t][:],
                    lhsT=GS[:, (sc0 + k) * elem:(sc0 + k + 1) * elem],
                    rhs=S[:, k * P:(k + 1) * P],
                    start=(bi == first_blk[t]),
                    stop=(bi == last_blk[t]),
                )
                if bi == last_blk[t]:
                    evict_fn(t, psum_tiles.pop(t))
        if post_call_hooks and ci in post_call_hooks:
            post_call_hooks[ci]()


def doff_scale_bcast(res, k0, nb, width):
    return res[:, k0:k0 + nb].to_broadcast([P, nb, width])


def build_bass(cfg, sched1, sched2, schedL, compile_=True):
    nc = bacc.Bacc("TRN2", target_bir_lowering=False, debug=False,
                   num_devices=NCORES, num_swdge_queues=4)
    IN, HID, LAT = cfg.IN, cfg.HID, cfg.LAT
    NPCP, TPC = cfg.NPCP, cfg.TPC

    x_tbl = nc.declare_dram_parameter("x_tbl", [cfg.XROWS, IN], BF16, isOutput=False)
    xT_c = nc.declare_dram_parameter("xT_c", [IN, NPCP], F32, isOutput=False)
    idx1 = nc.declare_dram_parameter("idx1", [P, sched1["totblk"] * 8], I16, isOutput=False)
    doff1 = nc.declare_dram_parameter("doff1", [P, sched1["totblk"]], BF16, isOutput=False)
    scal1 = nc.declare_dram_parameter("scal1", [P, sched1["totblk"]], BF16, isOutput=False)
    idx2 = nc.declare_dram_parameter("idx2", [P, sched2["totblk"] * 8], I16, isOutput=False)
    if schedL is not None:
        idxL = nc.declare_dram_parameter("idxL", [P, schedL["totblk"] * 8], I16, isOutput=False)
        doffL = nc.declare_dram_parameter("doffL", [P, schedL["totblk"]], BF16, isOutput=False)
        scalL = nc.declare_dram_parameter("scalL", [P, schedL["totblk"]], BF16, isOutput=False)
    doff2 = nc.declare_dram_parameter("doff2", [P, sched2["totblk"]], BF16, isOutput=False)
    scal2 = nc.declare_dram_parameter("scal2", [P, sched2["totblk"]], BF16, isOutput=False)
    w1lt = nc.declare_dram_parameter("w1lt", [IN, HID], F32, isOutput=False)
    w1rt = nc.declare_dram_parameter("w1rt", [IN, HID], F32, isOutput=False)
    b1 = nc.declare_dram_parameter("b1", [HID, 1], F32, isOutput=False)
    w2lt = nc.declare_dram_parameter("w2lt", [HID, LAT], F32, isOutput=False)
    w2rt = nc.declare_dram_parameter("w2rt", [HID, LAT], F32, isOutput=False)
    b2 = nc.declare_dram_parameter("b2", [LAT, 1], F32, isOutput=False)
    wdt = nc.declare_dram_parameter("wdt", [LAT, IN], F32, isOutput=False)
    bd = nc.declare_dram_parameter("bd", [IN, 1], F32, isOutput=False)
    iota_in = nc.declare_dram_parameter("iota_in", [P, P], BF16, isOutput=False)

    zT_out = nc.declare_dram_parameter("zT_out", [LAT, NPCP], F32, isOutput=True)
    xrT_out = nc.declare_dram_parameter("xrT_out", [IN, NPCP], F32, isOutput=True)

    p_self = nc.dram_tensor("p_self", [NPCP, LAT], F32)
    msg2loc = nc.dram_tensor("msg2loc", [LAT, NPCP], F32)
    p_full = nc.dram_tensor("p_full", [cfg.PROWS, LAT], F32, addr_space="Shared")
    s2_dram = nc.dram_tensor("s2_dram", [LAT, NPCP], F32)

    with TileContext(nc) as tc:
        with tc.tile_pool(name="const", bufs=1) as cpool, \
             tc.tile_pool(name="gpool", bufs=4) as gpool, \
             tc.tile_pool(name="spool", bufs=4) as spool, \
             tc.tile_pool(name="meta", bufs=1) as mpool, \
             tc.tile_pool(name="stage", bufs=3) as stpool, \
             tc.tile_pool(name="psum", bufs=8, space="PSUM") as psum_pool:

            nc.gpsimd.load_library(mlp)

            # constants
            iota_t = cpool.tile([P, P], BF16)
            nc.sync.dma_start(out=iota_t[:], in_=iota_in[:, :])
            w1lt_t = cpool.tile([IN, HID], F32)
            nc.sync.dma_start(out=w1lt_t[:], in_=w1lt[:, :])
            w1rt_t = cpool.tile([IN, HID], F32)
            nc.sync.dma_start(out=w1rt_t[:], in_=w1rt[:, :])
            b1_t = cpool.tile([HID, 1], F32)
            nc.sync.dma_start(out=b1_t[:], in_=b1[:, :])
            w2lt_t = cpool.tile([HID, LAT], F32)
            nc.sync.dma_start(out=w2lt_t[:], in_=w2lt[:, :])
            w2rt_t = cpool.tile([HID, LAT], F32)
            nc.sync.dma_start(out=w2rt_t[:], in_=w2rt[:, :])
            b2_t = cpool.tile([LAT, 1], F32)
            nc.sync.dma_start(out=b2_t[:], in_=b2[:, :])
            wdt_t = cpool.tile([LAT, IN], F32)
            nc.sync.dma_start(out=wdt_t[:], in_=wdt[:, :])
            bd_t = cpool.tile([IN, 1], F32)
            nc.sync.dma_start(out=bd_t[:], in_=bd[:, :])

            # resident metadata, layer 1
            idx1_t = mpool.tile([P, sched1["totblk"] * 8], I16, tag="idx")
            nc.sync.dma_start(out=idx1_t[:], in_=idx1[:, :])
            doff1_t = mpool.tile([P, sched1["totblk"]], BF16)
            nc.sync.dma_start(out=doff1_t[:], in_=doff1[:, :])
            scal1_t = mpool.tile([P, sched1["totblk"]], BF16)
            nc.sync.dma_start(out=scal1_t[:], in_=scal1[:, :])

            qstate = [0]

            def evict1(t, psum_t):
                c0, c1 = t * P, (t + 1) * P
                aggT = stpool.tile([IN, P], F32, tag="aggT")
                nc.vector.tensor_copy(out=aggT[:], in_=psum_t[:])
                xT_t = stpool.tile([IN, P], F32, tag="xTt")
                nc.sync.dma_start(out=xT_t[:], in_=xT_c[:, c0:c1])
                hps = psum_pool.tile([HID, P], F32, tag="seg")
                nc.tensor.matmul(hps[:], lhsT=w1lt_t[:], rhs=aggT[:],
                                 start=True, stop=False)
                nc.tensor.matmul(hps[:], lhsT=w1rt_t[:], rhs=xT_t[:],
                                 start=False, stop=True)
                hT = stpool.tile([HID, P], F32, tag="hT")
                nc.scalar.activation(hT[:], hps[:],
                                     mybir.ActivationFunctionType.Relu,
                                     bias=b1_t[:])
                pns = psum_pool.tile([P, LAT], F32, tag="seg")
                nc.tensor.matmul(pns[:], lhsT=hT[:], rhs=w2lt_t[:],
                                 start=True, stop=True)
                s2s = psum_pool.tile([LAT, P], F32, tag="seg")
                nc.tensor.matmul(s2s[:], lhsT=w2rt_t[:], rhs=hT[:],
                                 start=True, stop=True)
                pn = stpool.tile([P, LAT], F32, tag="pn")
                nc.vector.tensor_copy(out=pn[:], in_=pns[:])
                s2 = stpool.tile([LAT, P], F32, tag="s2")
                nc.vector.tensor_scalar_add(s2[:], s2s[:], b2_t[:])
                nc.sync.dma_start(out=p_self[c0:c1, :], in_=pn[:])
                nc.sync.dma_start(out=s2_dram[:, c0:c1], in_=s2[:])

            emit_gather_segsum(nc, tc, (gpool, spool), cfg, sched1, IN,
                               x_tbl, idx1_t[:], doff1_t[:], scal1_t[:],
                               psum_pool, (IN, P), evict1, qstate, iota_t,
                               gdt=BF16)

            # AllGather p (trigger waits on p_self, then CC runs in
            # parallel with the local-edge gather phase below)
            nc.gpsimd.collective_compute(
                "AllGather", mybir.AluOpType.bypass,
                replica_groups=[list(range(NCORES))],
                ins=[p_self.ap().opt()],
                outs=[p_full.ap().opt()],
            )

            # local-edge gather phase: own-core sources from p_self, no AG dep
            if schedL is not None:
                idxL_t = mpool.tile([P, schedL["totblk"] * 8], I16)
                nc.sync.dma_start(out=idxL_t[:], in_=idxL[:, :])
                doffL_t = mpool.tile([P, schedL["totblk"]], BF16)
                nc.sync.dma_start(out=doffL_t[:], in_=doffL[:, :])
                scalL_t = mpool.tile([P, schedL["totblk"]], BF16)
                nc.sync.dma_start(out=scalL_t[:], in_=scalL[:, :])

                def evict_loc(t, psum_t):
                    c0, c1 = t * P, (t + 1) * P
                    locT = stpool.tile([LAT, P], F32, tag="locT")
                    nc.vector.tensor_copy(out=locT[:], in_=psum_t[:])
                    nc.sync.dma_start(out=msg2loc[:, c0:c1], in_=locT[:])

                emit_gather_segsum(nc, tc, (gpool, spool), cfg, schedL, LAT,
                                   p_self, idxL_t[:], doffL_t[:], scalL_t[:],
                                   psum_pool, (LAT, P), evict_loc, qstate,
                                   iota_t)

            # resident metadata, layer 2
            idx2_t = mpool.tile([P, sched2["totblk"] * 8], I16, tag="idx")
            nc.sync.dma_start(out=idx2_t[:], in_=idx2[:, :])
            doff2_t = mpool.tile([P, sched2["totblk"]], BF16)
            nc.sync.dma_start(out=doff2_t[:], in_=doff2[:, :])
            scal2_t = mpool.tile([P, sched2["totblk"]], BF16)
            nc.sync.dma_start(out=scal2_t[:], in_=scal2[:, :])

            def evict2(t, psum_t):
                c0, c1 = t * P, (t + 1) * P
                s2t = stpool.tile([LAT, P], F32, tag="s2t")
                nc.sync.dma_start(out=s2t[:], in_=s2_dram[:, c0:c1])
                zT = stpool.tile([LAT, P], F32, tag="zT")
                nc.vector.tensor_add(out=zT[:], in0=psum_t[:], in1=s2t[:])
                if schedL is not None:
                    loct = stpool.tile([LAT, P], F32, tag="loct")
                    nc.sync.dma_start(out=loct[:], in_=msg2loc[:, c0:c1])
                    nc.vector.tensor_add(out=zT[:], in0=zT[:], in1=loct[:])
                xrp = psum_pool.tile([IN, P], F32, tag="seg")
                nc.tensor.matmul(xrp[:], lhsT=wdt_t[:], rhs=zT[:],
                                 start=True, stop=True)
                xr = stpool.tile([IN, P], F32, tag="xr")
                nc.vector.tensor_scalar_add(xr[:], xrp[:], bd_t[:])
                nc.sync.dma_start(out=zT_out[:, c0:c1], in_=zT[:])
                nc.sync.dma_start(out=xrT_out[:, c0:c1], in_=xr[:])

            emit_gather_segsum(nc, tc, (gpool, spool), cfg, sched2, LAT,
                               p_full, idx2_t[:], doff2_t[:], scal2_t[:],
                               psum_pool, (LAT, P), evict2, qstate, iota_t)

    if compile_:
        nc.compile()
    return nc


# ------------------------------------------------------------ host driver

def prepare(x, edge_index, W1_l, b1_, W1_r, W2_l, b2_, W2_r, W_dec, b_dec,
            cfg):
    src = np.asarray(edge_index[0], np.int64)
    dst = np.asarray(edge_index[1], np.int64)
    N, E = cfg.N, cfg.E
    deg = np.bincount(dst, minlength=N).astype(np.float32)
    deginv = 1.0 / np.maximum(deg, 1.0)

    owner = dst // cfg.NPC
    src_by, dstl_by, dgi_by = [], [], []
    for c in range(NCORES):
        m = np.nonzero(owner == c)[0]
        src_by.append(src[m])
        dstl_by.append(dst[m] - c * cfg.NPC)
        dgi_by.append(deginv[dst[m]])

    sched1, pc1 = build_layer_schedule(cfg, src_by, dstl_by, dgi_by, cfg.XROWS)

    # layer-2 local/remote split: edges whose src this core owns can gather
    # from p_self before the AllGather completes.  Per-tile local quotas come
    # from the cross-core min (rounded to the block grid) to minimize padding.
    TPC = cfg.TPC
    locm_by = [(src_by[c] // cfg.NPC) == c for c in range(NCORES)]
    tile_by = [dstl_by[c] // P for c in range(NCORES)]
    loc_cnt = np.zeros((NCORES, TPC), np.int64)
    for c in range(NCORES):
        np.add.at(loc_cnt[c], tile_by[c][locm_by[c]], 1)
    qloc = np.zeros(TPC, np.int64)  # local phase disabled (hole inflation)
    srcL_by, dstlL_by, dgiL_by = [], [], []
    src2_by, dstlR_by, dgiR_by = [], [], []
    for c in range(NCORES):
        sel = np.zeros(len(src_by[c]), bool)
        for t in range(TPC):
            e = np.nonzero(locm_by[c] & (tile_by[c] == t))[0]
            take = min(len(e), int(qloc[t]))
            sel[e[:take]] = True
        srcL_by.append(src_by[c][sel] - c * cfg.NPC)
        dstlL_by.append(dstl_by[c][sel])
        dgiL_by.append(dgi_by[c][sel])
        r = ~sel
        so = src_by[c][r] // cfg.NPC
        src2_by.append(so * cfg.NPCP + (src_by[c][r] - so * cfg.NPC))
        dstlR_by.append(dstl_by[c][r])
        dgiR_by.append(dgi_by[c][r])

    use_local = int(qloc.sum()) > 0
    if use_local:
        schedL, pcL = build_layer_schedule(cfg, srcL_by, dstlL_by, dgiL_by,
                                           cfg.NPCP, nseg=1)
    else:
        schedL, pcL = None, None
    sched2, pc2 = build_layer_schedule(cfg, src2_by, dstlR_by, dgiR_by,
                                       cfg.PROWS)

    iota = np.broadcast_to(np.arange(P), (P, P)).astype(BF)
    shared = dict(
        x_tbl=np.ascontiguousarray(np.asarray(x, np.float32).astype(BF)),
        w1lt=np.ascontiguousarray(W1_l.T, np.float32),
        w1rt=np.ascontiguousarray(W1_r.T, np.float32),
        b1=np.ascontiguousarray(b1_.reshape(-1, 1), np.float32),
        w2lt=np.ascontiguousarray(W2_l.T, np.float32),
        w2rt=np.ascontiguousarray(W2_r.T, np.float32),
        b2=np.ascontiguousarray(b2_.reshape(-1, 1), np.float32),
        wdt=np.ascontiguousarray(W_dec.T, np.float32),
        bd=np.ascontiguousarray(b_dec.reshape(-1, 1), np.float32),
        iota_in=iota,
    )
    in_maps = []
    for c in range(NCORES):
        xs = np.zeros((cfg.IN, cfg.NPCP), np.float32)
        xs[:, :cfg.NPC] = x[c * cfg.NPC:(c + 1) * cfg.NPC].T
        m = dict(shared)
        m.update(
            xT_c=xs,
            idx1=pc1[c]["idx16"], doff1=pc1[c]["dstoff"].astype(BF),
            scal1=pc1[c]["scale"].astype(BF),
            idx2=pc2[c]["idx16"], doff2=pc2[c]["dstoff"].astype(BF),
            scal2=pc2[c]["scale"].astype(BF),
            **({"idxL": pcL[c]["idx16"], "doffL": pcL[c]["dstoff"].astype(BF),
                "scalL": pcL[c]["scale"].astype(BF)} if pcL else {}),
        )
        in_maps.append(m)
    return sched1, sched2, schedL, in_maps


def assemble(results, cfg):
    xr = np.empty((cfg.N, cfg.IN), np.float32)
    z = np.empty((cfg.N, cfg.LAT), np.float32)
    for c in range(NCORES):
        xr[c * cfg.NPC:(c + 1) * cfg.NPC] = results[c]["xrT_out"][:, :cfg.NPC].T
        z[c * cfg.NPC:(c + 1) * cfg.NPC] = results[c]["zT_out"][:, :cfg.NPC].T
    return xr, z


def kernel(x, edge_index, W1_l, b1_l, W1_r, W2_l, b2_l, W2_r, W_dec, b_dec,
           _trace=False):
    x = np.asarray(x)
    edge_index = np.asarray(edge_index)
    n, e = x.shape[0], edge_index.shape[1]
    cfg = Cfg(n, e, x.shape[1], W1_l.shape[0], W2_l.shape[0],
              n // NCORES, gsize=7 if n == 100000 else 2)
    sched1, sched2, schedL, in_maps = prepare(
        x, edge_index, np.asarray(W1_l), np.asarray(b1_l), np.asarray(W1_r),
        np.asarray(W2_l), np.asarray(b2_l), np.asarray(W2_r),
        np.asarray(W_dec), np.asarray(b_dec), cfg)

    nc = build_bass(cfg, sched1, sched2, schedL)
    res = run_bass_kernel_spmd(nc, in_maps, core_ids=list(range(NCORES)),
                               trace=_trace)
    xr, z = assemble(res.results, cfg)
    kernel.last_exec_ns = res.exec_time_ns
    return (xr, z)
